# revision 17
# baseline (speedup 1.0000x reference)
"""Trainium2 Bass kernel for DepthAdapterWindowAttn.

Math (per batch image, H=W=128, C=106 feat channels):
  feat = concat(codes, depth)                              # (N, 106)
  s    = feat @ gate_w            (gate bias dropped: softmax-invariant)
  E    = exp(s)                   (no max-subtract needed: |s| ~ N(0,1))
  p    = feat @ Wproj + b         (proj of a shifted window == shift of proj)
  F    = [E*p ; E]                # 107 channels
  G    = box3x3_reflect(F)        # separable: W-pass then H-pass
  attended = G[0:106] / G[106]    # softmax-weighted window sum
  y1 = attended @ W1 + b1 ; x1 = relu(LN(y1))
  y2 = x1 @ W2 + b2       ; x2 = relu(LN(y2))
  out = codes + x2 @ Wout + bout

Key trick: LayerNorm is invariant to a positive per-pixel scale, so the
division by Z = G[106] cancels inside LN1:  mm1 consumes the *unnormalized*
box output G directly, with w1_aug row 106 (= b1) multiplied by the Z row,
which scales the bias by exactly the right factor.  The softmax denominator
is never divided out explicitly.

Sharding: data-parallel over batch B=8, one image per NeuronCore.
"""

import numpy as np

import concourse.bacc as bacc
import concourse.bass as bass
import concourse.mybir as mybir
import concourse.tile as tile
from concourse.bass_utils import run_bass_kernel_spmd
from concourse.masks import make_identity

F32 = mybir.dt.float32
BF16 = mybir.dt.bfloat16
AF = mybir.ActivationFunctionType
ALU = mybir.AluOpType

H = 128
W = 128
NPIX = H * W            # 16384
CD = 90                 # code dim
DD = 16                 # depth dim
C = CD + DD             # 106
CA = C + 1              # 107 (augmented with ones/E row)
HID = 384
EPS = 1e-5
NCHUNK = NPIX // 512    # 32
NBLK = NPIX // 128      # 128
GRP = 16                # LN stat batching group


def _consts(nc, tc, consts, dram, apply_ln_affine):
    """Load/cast all weights into SBUF bf16 tiles."""
    i128b = consts.tile([128, 128], BF16, tag="i128b")
    make_identity(nc, i128b)

    ones1 = consts.tile([1, 128], BF16, tag="ones1")
    nc.vector.memset(ones1, 1.0)

    eps_t = consts.tile([128, 1], F32, tag="eps_t")
    nc.vector.memset(eps_t, EPS)

    def staged(name, shape_dst, fill_zero, loads, dtype=BF16):
        stg = consts.tile(shape_dst, F32, tag=f"stg_{name}")
        if fill_zero:
            nc.vector.memset(stg, 0.0)
        for dst_sl, src_ap in loads:
            nc.sync.dma_start(out=stg[dst_sl], in_=src_ap)
        t = consts.tile(shape_dst, dtype, tag=name)
        nc.vector.tensor_copy(t, stg)
        return t

    def bcast_ap(handle, n):
        ap = handle[:]
        return bass.AP(tensor=ap.tensor, offset=ap.offset, ap=[[0, 128], [1, n]])

    k = {}
    # Wpg_aug[kin, mout]: kin 106 = ones row, mout 106 = E pass-through
    wpg = staged(
        "wpg", [CA, CA], True,
        [((slice(0, C), slice(0, C)), dram["attn_proj_w"][:, :]),
         ((slice(C, CA), slice(0, C)), dram["attn_proj_b"][None, :])])
    # E pass-through column: wpg[:, 106] = e_106, taken from the identity
    # (avoids a single-partition write at partition 106, which BIR rejects)
    nc.vector.tensor_copy(wpg[0:CA, C:C + 1], i128b[0:CA, C:C + 1])
    k["wpg"] = wpg

    k["gw_rep"] = staged(
        "gw_rep", [128, CA], True,
        [((slice(0, 128), slice(0, C)), bcast_ap(dram["attn_gate_w"], C))],
        dtype=F32)

    k["w1a"] = staged(
        "w1a", [CA, HID], False,
        [((slice(0, C), slice(0, HID)), dram["mlp_w1"][:, :]),
         ((slice(C, CA), slice(0, HID)), dram["mlp_b1"][None, :])])

    w2b_stg = consts.tile([128, 3, HID], F32, tag="w2stg")
    for kb in range(3):
        nc.sync.dma_start(out=w2b_stg[:, kb, :],
                          in_=dram["mlp_w2"][kb * 128:(kb + 1) * 128, :])
    w2b = consts.tile([128, 3, HID], BF16, tag="w2b")
    nc.vector.tensor_copy(w2b, w2b_stg)
    k["w2b"] = w2b

    wob_stg = consts.tile([128, 3, CD], F32, tag="wostg")
    for kb in range(3):
        nc.sync.dma_start(out=wob_stg[:, kb, :],
                          in_=dram["out_w"][kb * 128:(kb + 1) * 128, :])
    wob = consts.tile([128, 3, CD], BF16, tag="wob")
    nc.vector.tensor_copy(wob, wob_stg)
    k["wob"] = wob

    k["b2_row"] = staged("b2r", [1, HID], False,
                         [((slice(0, 1), slice(0, HID)), dram["mlp_b2"][None, :])])
    k["ob_row"] = staged("obr", [1, CD], False,
                         [((slice(0, 1), slice(0, CD)), dram["out_b"][None, :])])

    if apply_ln_affine:
        k["g1_rep"] = staged("g1r", [128, HID], False,
                             [((slice(0, 128), slice(0, HID)), bcast_ap(dram["ln1_g"], HID))])
        k["b1_rep"] = staged("b1r", [128, HID], False,
                             [((slice(0, 128), slice(0, HID)), bcast_ap(dram["ln1_b"], HID))])
        k["g2_rep"] = staged("g2r", [128, HID], False,
                             [((slice(0, 128), slice(0, HID)), bcast_ap(dram["ln2_g"], HID))])
        k["b2l_rep"] = staged("b2lr", [128, HID], False,
                              [((slice(0, 128), slice(0, HID)), bcast_ap(dram["ln2_b"], HID))])
    k["i128b"] = i128b
    k["ones1"] = ones1
    k["eps_t"] = eps_t
    return k


def build_kernel(apply_ln_affine: bool) -> bass.Bass:
    nc = bacc.Bacc("TRN2", target_bir_lowering=False, num_devices=8)

    dram = {}
    dram["codes"] = nc.declare_dram_parameter("codes", [NPIX, CD], F32, isOutput=False)
    dram["depth"] = nc.declare_dram_parameter("depth", [NPIX, DD], F32, isOutput=False)
    for name, shape in [
        ("attn_proj_w", [C, C]), ("attn_proj_b", [C]), ("attn_gate_w", [C, 1]),
        ("mlp_w1", [C, HID]), ("mlp_b1", [HID]), ("ln1_g", [HID]), ("ln1_b", [HID]),
        ("mlp_w2", [HID, HID]), ("mlp_b2", [HID]), ("ln2_g", [HID]), ("ln2_b", [HID]),
        ("out_w", [HID, CD]), ("out_b", [CD]),
    ]:
        dram[name] = nc.declare_dram_parameter(name, shape, F32, isOutput=False)
    out = nc.declare_dram_parameter("out", [NPIX, CD], F32, isOutput=True)
    codes = dram["codes"]
    depth = dram["depth"]

    with tile.TileContext(nc) as tc:
        with (
            tc.tile_pool(name="consts", bufs=1) as consts,
            tc.tile_pool(name="fields", bufs=1) as fields,
            tc.tile_pool(name="px", bufs=GRP + 2) as pxp,
            tc.tile_pool(name="uchunk", bufs=3) as uchp,
            tc.tile_pool(name="scrap", bufs=3) as scrapp,
            tc.tile_pool(name="ysb", bufs=GRP + 2) as ysbp,
            tc.tile_pool(name="xn", bufs=3) as xnp,
            tc.tile_pool(name="xt", bufs=3) as xtp,
            tc.tile_pool(name="stats", bufs=2) as statsp,
            tc.tile_pool(name="outp", bufs=4) as outp,
        ):
            k = _consts(nc, tc, consts, dram, apply_ln_affine)
            i128b, ones1, eps_t = k["i128b"], k["ones1"], k["eps_t"]

            # x = h*128 + w pixel flattening; free-dim pads for box shifts
            F_f = fields.tile([CA, NPIX + 2], BF16, tag="F_f")    # center off 1
            RW_f = fields.tile([CA, NPIX + 256], BF16, tag="RW_f")  # center off 128
            G_f = fields.tile([CA, NPIX], BF16, tag="G_f")
            s2dw = fields.tile([128, 128], F32, tag="s2dw")       # s[w, h]
            E2dw = fields.tile([128, 128], F32, tag="E2dw")
            nc.vector.memset(F_f[:, 0:1], 0.0)
            nc.vector.memset(F_f[:, NPIX + 1:NPIX + 2], 0.0)
            nc.vector.memset(RW_f[:, 0:128], 0.0)
            nc.vector.memset(RW_f[:, NPIX + 128:NPIX + 256], 0.0)
            Fc = F_f[:, 1:1 + NPIX]
            RWc = RW_f[:, 128:128 + NPIX]

            # ---- phase A: load, gate dot, E, u = E*feat, transpose-in, proj
            with (
                tc.tile_pool(name="ps_t", bufs=3, space="PSUM") as ps_t,
                tc.tile_pool(name="ps_f", bufs=2, space="PSUM") as ps_f,
            ):
                for g in range(NBLK // GRP):
                    pxs = []
                    for j in range(GRP):
                        b = g * GRP + j
                        px = pxp.tile([128, CA], F32, tag="px")
                        nc.sync.dma_start(out=px[:, 0:CD],
                                          in_=codes[b * 128:(b + 1) * 128, :])
                        nc.sync.dma_start(out=px[:, CD:C],
                                          in_=depth[b * 128:(b + 1) * 128, :])
                        nc.vector.memset(px[:, C:CA], 1.0)
                        scr = scrapp.tile([128, CA], F32, tag="sscr")
                        nc.vector.tensor_mul(scr, px, k["gw_rep"])
                        nc.vector.reduce_sum(s2dw[:, b:b + 1], scr,
                                             mybir.AxisListType.X)
                        pxs.append(px)
                    nc.scalar.activation(
                        out=E2dw[:, g * GRP:(g + 1) * GRP],
                        in_=s2dw[:, g * GRP:(g + 1) * GRP], func=AF.Exp)
                    for j in range(GRP):
                        b = g * GRP + j
                        upx = pxp.tile([128, CA], BF16, tag="upx")
                        nc.scalar.activation(out=upx, in_=pxs[j], func=AF.Copy,
                                             scale=E2dw[:, b:b + 1])
                        tp = ps_t.tile([CA, 128], BF16, tag="tp")
                        nc.tensor.transpose(tp, upx, i128b)
                        if b % 4 == 0:
                            uch = uchp.tile([CA, 512], BF16, tag="uc")
                        if b % 2 == 0:
                            nc.vector.tensor_copy(
                                uch[:, (b % 4) * 128:(b % 4 + 1) * 128], tp)
                        else:
                            nc.scalar.copy(
                                uch[:, (b % 4) * 128:(b % 4 + 1) * 128], tp)
                        if b % 4 == 3:
                            c = b // 4
                            fps = ps_f.tile([CA, 512], F32, tag="fps")
                            nc.tensor.matmul(fps, lhsT=k["wpg"], rhs=uch,
                                             start=True, stop=True)
                            if c % 2 == 0:
                                nc.vector.tensor_copy(Fc[:, c * 512:(c + 1) * 512], fps)
                            else:
                                nc.scalar.copy(Fc[:, c * 512:(c + 1) * 512], fps)

            # ---- phase B: separable 3x3 box with reflect boundary
            for c in range(NCHUNK):
                sl = slice(c * 512, (c + 1) * 512)
                t = scrapp.tile([CA, 512], BF16, tag="boxt")
                nc.vector.tensor_add(t, F_f[:, c * 512:c * 512 + 512],
                                     F_f[:, c * 512 + 2:c * 512 + 514])
                nc.vector.tensor_add(RWc[:, sl], t, Fc[:, sl])
            Fv = Fc.rearrange("p (h w) -> p h w", h=H)
            RWv = RWc.rearrange("p (h w) -> p h w", h=H)
            nc.vector.scalar_tensor_tensor(
                out=RWv[:, :, 0:1], in0=Fv[:, :, 1:2], scalar=2.0,
                in1=Fv[:, :, 0:1], op0=ALU.mult, op1=ALU.add)
            nc.vector.scalar_tensor_tensor(
                out=RWv[:, :, 127:128], in0=Fv[:, :, 126:127], scalar=2.0,
                in1=Fv[:, :, 127:128], op0=ALU.mult, op1=ALU.add)
            for c in range(NCHUNK):
                sl = slice(c * 512, (c + 1) * 512)
                t = scrapp.tile([CA, 512], BF16, tag="boxt")
                nc.vector.tensor_add(t, RW_f[:, c * 512:c * 512 + 512],
                                     RW_f[:, c * 512 + 256:c * 512 + 768])
                nc.vector.tensor_add(G_f[:, sl], t, RWc[:, sl])
            nc.vector.scalar_tensor_tensor(
                out=G_f[:, 0:128], in0=RWc[:, 128:256], scalar=2.0,
                in1=RWc[:, 0:128], op0=ALU.mult, op1=ALU.add)
            nc.vector.scalar_tensor_tensor(
                out=G_f[:, NPIX - 128:NPIX], in0=RWc[:, NPIX - 256:NPIX - 128],
                scalar=2.0, in1=RWc[:, NPIX - 128:NPIX], op0=ALU.mult, op1=ALU.add)

            # ---- phase C: MLPs + LNs + residual
            def ln_rstd_batched(mv):
                """mv [128, GRP, 2] (mean, var) -> rstd [128, GRP]."""
                sd = statsp.tile([128, GRP], F32, tag="sd")
                nc.scalar.activation(out=sd, in_=mv[:, :, 1], func=AF.Sqrt,
                                     bias=eps_t, scale=1.0)
                rstd = statsp.tile([128, GRP], F32, tag="rstd")
                nc.vector.reciprocal(rstd, sd)
                return rstd

            def neg_mu_rstd(mv, rstd):
                """[128, GRP] tile of -mean * rstd (fused LN bias)."""
                nm = statsp.tile([128, GRP], F32, tag="nm")
                nc.vector.scalar_tensor_tensor(
                    out=nm, in0=mv[:, :, 0], scalar=-1.0, in1=rstd,
                    op0=ALU.mult, op1=ALU.mult)
                return nm

            def apply_ln_relu(xn, y_sb, mv, nm, rstd, j, gamma_rep, beta_rep):
                """x = relu(LN(y)): one fused scalar-engine op when the LN
                affine is identity (relu commutes with the positive rstd
                scale: Relu(y*rstd - mu*rstd) == relu(LN(y)))."""
                if gamma_rep is None:
                    nc.scalar.activation(
                        out=xn, in_=y_sb, func=AF.Relu,
                        bias=nm[:, j:j + 1], scale=rstd[:, j:j + 1])
                else:
                    nc.vector.tensor_scalar(
                        out=xn, in0=y_sb, scalar1=mv[:, j, 0:1],
                        scalar2=rstd[:, j:j + 1],
                        op0=ALU.subtract, op1=ALU.mult)
                    nc.vector.tensor_mul(xn, xn, gamma_rep)
                    nc.vector.tensor_add(xn, xn, beta_rep)
                    nc.scalar.activation(out=xn, in_=xn, func=AF.Relu)

            with (
                tc.tile_pool(name="ps_y", bufs=2, space="PSUM") as ps_y,
                tc.tile_pool(name="ps_xt", bufs=2, space="PSUM") as ps_xt,
                tc.tile_pool(name="ps_o", bufs=2, space="PSUM") as ps_o,
            ):
                def transpose_copy(xn):
                    """xn (already relu'd) -> 3 transposed [128,128] tiles."""
                    xt = xtp.tile([128, 3, 128], BF16, tag="xt")
                    tps = ps_xt.tile([128, 3, 128], BF16, tag="tps")
                    for kb in range(3):
                        nc.tensor.transpose(tps[:, kb, :],
                                            xn[:, kb * 128:(kb + 1) * 128], i128b)
                        if kb == 1:
                            nc.scalar.copy(xt[:, kb, :], tps[:, kb, :])
                        else:
                            nc.vector.tensor_copy(xt[:, kb, :], tps[:, kb, :])
                    return xt

                for g in range(NBLK // GRP):
                    mv1 = statsp.tile([128, GRP, 2], F32, tag="mv1")
                    y1s = []
                    for j in range(GRP):
                        b = g * GRP + j
                        yps = ps_y.tile([128, HID], F32, tag="yps")
                        nc.tensor.matmul(yps, lhsT=G_f[:, b * 128:(b + 1) * 128],
                                         rhs=k["w1a"], start=True, stop=True)
                        y_sb = ysbp.tile([128, HID], BF16, tag="y1sb")
                        nc.scalar.copy(y_sb, yps)
                        st = scrapp.tile([128, 6], F32, tag="st")
                        nc.vector.bn_stats(out=st, in_=yps)
                        nc.vector.bn_aggr(out=mv1[:, j, :], in_=st)
                        y1s.append(y_sb)
                    rstd1 = ln_rstd_batched(mv1)
                    nm1 = neg_mu_rstd(mv1, rstd1)

                    mv2 = statsp.tile([128, GRP, 2], F32, tag="mv2")
                    y2s = []
                    for j in range(GRP):
                        b = g * GRP + j
                        xn = xnp.tile([128, HID], BF16, tag="x1n")
                        apply_ln_relu(xn, y1s[j], mv1, nm1, rstd1, j,
                                      k.get("g1_rep"), k.get("b1_rep"))
                        xt = transpose_copy(xn)
                        yps = ps_y.tile([128, HID], F32, tag="yps")
                        for kb in range(3):
                            nc.tensor.matmul(yps, lhsT=xt[:, kb, :],
                                             rhs=k["w2b"][:, kb, :],
                                             start=(kb == 0), stop=False)
                        nc.tensor.matmul(yps, lhsT=ones1, rhs=k["b2_row"],
                                         start=False, stop=True)
                        y_sb = ysbp.tile([128, HID], BF16, tag="y2sb")
                        nc.scalar.copy(y_sb, yps)
                        st = scrapp.tile([128, 6], F32, tag="st")
                        nc.vector.bn_stats(out=st, in_=yps)
                        nc.vector.bn_aggr(out=mv2[:, j, :], in_=st)
                        y2s.append(y_sb)
                    rstd2 = ln_rstd_batched(mv2)
                    nm2 = neg_mu_rstd(mv2, rstd2)

                    for j in range(GRP):
                        b = g * GRP + j
                        xn = xnp.tile([128, HID], BF16, tag="x2n")
                        apply_ln_relu(xn, y2s[j], mv2, nm2, rstd2, j,
                                      k.get("g2_rep"), k.get("b2l_rep"))
                        xt = transpose_copy(xn)
                        ops = ps_o.tile([128, CD], F32, tag="ops")
                        for kb in range(3):
                            nc.tensor.matmul(ops, lhsT=xt[:, kb, :],
                                             rhs=k["wob"][:, kb, :],
                                             start=(kb == 0), stop=False)
                        nc.tensor.matmul(ops, lhsT=ones1, rhs=k["ob_row"],
                                         start=False, stop=True)
                        cb = outp.tile([128, CD], F32, tag="cb")
                        nc.sync.dma_start(out=cb,
                                          in_=codes[b * 128:(b + 1) * 128, :])
                        ot = outp.tile([128, CD], F32, tag="ot")
                        nc.vector.tensor_add(ot, ops, cb)
                        nc.sync.dma_start(out=out[b * 128:(b + 1) * 128, :], in_=ot)

    nc.compile()
    return nc


def build_kernel_v2() -> bass.Bass:
    """Channel-major phase C: no per-block transposes, matmul-based LN stats.

    Tricks on top of v1:
      - Wpg folds into W1:  y1 = (Wpg @ W1aug).T @ box(u), so the proj matmul
        and the F field disappear; the box filter runs directly on
        u = [E*feat ; E].
      - Column-centered weights: subtracting each row's mean over the output
        dim from Wc / W2 / b2 makes mean(y) == 0 exactly, killing the LN mean
        computation (LN is then y * rstd).
      - Channel-major mm1/mm2: LN variance via sum of squares over the
        partition dim = ones-vector matmuls; var1 via the quadratic form
        B.T (Wc' Wc'.T) B (one [107x107] matmul + one dot pass).
      - Deferred normalization: x1 = relu(y1') unscaled (LN2 is invariant to
        positive per-pixel scale); b2 rides a rank-1 matmul scaled by std1;
        the exact rstd2 scale is applied pixel-major on the 90-dim output.
    """
    nc = bacc.Bacc("TRN2", target_bir_lowering=False, num_devices=8)

    dram = {}
    dram["codes"] = nc.declare_dram_parameter("codes", [NPIX, CD], F32, isOutput=False)
    dram["depth"] = nc.declare_dram_parameter("depth", [NPIX, DD], F32, isOutput=False)
    for name, shape in [
        ("attn_proj_w", [C, C]), ("attn_proj_b", [C]), ("attn_gate_w", [C, 1]),
        ("mlp_w1", [C, HID]), ("mlp_b1", [HID]), ("ln1_g", [HID]), ("ln1_b", [HID]),
        ("mlp_w2", [HID, HID]), ("mlp_b2", [HID]), ("ln2_g", [HID]), ("ln2_b", [HID]),
        ("out_w", [HID, CD]), ("out_b", [CD]),
    ]:
        dram[name] = nc.declare_dram_parameter(name, shape, F32, isOutput=False)
    out = nc.declare_dram_parameter("out", [NPIX, CD], F32, isOutput=True)
    codes = dram["codes"]
    depth = dram["depth"]
    INV_HID = 1.0 / HID
    SQRT_INV_HID = float(np.sqrt(1.0 / HID))

    with tile.TileContext(nc) as tc:
        with (
            tc.tile_pool(name="consts", bufs=1) as consts,
            tc.tile_pool(name="fields", bufs=1) as fields,
            tc.tile_pool(name="px", bufs=GRP + 2) as pxp,
            tc.tile_pool(name="scrap", bufs=4) as scrapp,
            tc.tile_pool(name="xs", bufs=9) as xsp,
            tc.tile_pool(name="sq", bufs=6) as sqp,
            tc.tile_pool(name="prod", bufs=3) as prodp,
            tc.tile_pool(name="rows", bufs=4) as rowsp,
            tc.tile_pool(name="outp", bufs=8) as outp,
        ):
            # ---- constants / weight prep
            i128b = consts.tile([128, 128], BF16, tag="i128b")
            make_identity(nc, i128b)
            ones_col = consts.tile([128, 1], BF16, tag="ones_col")
            nc.vector.memset(ones_col, 1.0)
            inv_col = consts.tile([128, 1], BF16, tag="inv_col")
            nc.vector.memset(inv_col, INV_HID)
            eps_t = consts.tile([128, 1], F32, tag="eps_t")
            nc.vector.memset(eps_t, EPS)

            def staged(name, shape_dst, fill_zero, loads, dtype=BF16):
                stg = consts.tile(shape_dst, F32, tag=f"stg_{name}")
                if fill_zero:
                    nc.vector.memset(stg, 0.0)
                for dst_sl, src_ap in loads:
                    nc.sync.dma_start(out=stg[dst_sl], in_=src_ap)
                t = consts.tile(shape_dst, dtype, tag=name)
                nc.vector.tensor_copy(t, stg)
                return t

            def bcast_ap(handle, n):
                ap = handle[:]
                return bass.AP(tensor=ap.tensor, offset=ap.offset,
                               ap=[[0, 128], [1, n]])

            wpg = staged(
                "wpg", [CA, CA], True,
                [((slice(0, C), slice(0, C)), dram["attn_proj_w"][:, :]),
                 ((slice(C, CA), slice(0, C)), dram["attn_proj_b"][None, :])])
            nc.vector.tensor_copy(wpg[0:CA, C:C + 1], i128b[0:CA, C:C + 1])

            gw_rep = staged(
                "gw_rep", [128, CA], True,
                [((slice(0, 128), slice(0, C)), bcast_ap(dram["attn_gate_w"], C))],
                dtype=F32)

            w1a = staged(
                "w1a", [CA, HID], False,
                [((slice(0, C), slice(0, HID)), dram["mlp_w1"][:, :]),
                 ((slice(C, CA), slice(0, HID)), dram["mlp_b1"][None, :])])

            with tc.tile_pool(name="ps_prep", bufs=2, space="PSUM") as ps_prep:
                # Wc = rowcenter(Wpg @ W1aug)  [CA, HID]
                tp_ps = ps_prep.tile([128, 128], BF16, tag="prepb")
                nc.tensor.transpose(tp_ps[0:CA, 0:CA], wpg, i128b[0:CA, 0:CA])
                wpgT = consts.tile([CA, CA], BF16, tag="wpgT")
                nc.vector.tensor_copy(wpgT, tp_ps[0:CA, 0:CA])
                cb_ps = ps_prep.tile([128, 512], F32, tag="prepf")
                nc.tensor.matmul(cb_ps[0:CA, 0:HID], lhsT=wpgT, rhs=w1a,
                                 start=True, stop=True)
                rs = consts.tile([CA, 1], F32, tag="rs")
                nc.vector.reduce_sum(rs, cb_ps[0:CA, 0:HID], mybir.AxisListType.X)
                rm = consts.tile([CA, 1], F32, tag="rm")
                nc.vector.tensor_scalar_mul(rm, rs, INV_HID)
                wc = consts.tile([CA, HID], BF16, tag="wc")
                nc.vector.tensor_scalar_sub(wc, cb_ps[0:CA, 0:HID], rm)

                # M1 = Wc @ Wc.T  [CA, CA]
                wcT = consts.tile([128, 3, CA], BF16, tag="wcT")
                for kb in range(3):
                    t_ps = ps_prep.tile([128, 128], BF16, tag="prepb")
                    nc.tensor.transpose(t_ps[0:128, 0:CA],
                                        wc[:, kb * 128:(kb + 1) * 128],
                                        i128b[0:CA, 0:CA])
                    nc.vector.tensor_copy(wcT[:, kb, :], t_ps[0:128, 0:CA])
                m1_ps = ps_prep.tile([128, 512], F32, tag="prepf")
                for kb in range(3):
                    nc.tensor.matmul(m1_ps[0:CA, 0:CA], lhsT=wcT[:, kb, :],
                                     rhs=wcT[:, kb, :], start=(kb == 0),
                                     stop=(kb == 2))
                m1 = consts.tile([CA, CA], BF16, tag="m1")
                nc.vector.tensor_copy(m1, m1_ps[0:CA, 0:CA])

            # W2 row-centered over output dim, [128, 3, HID]
            w2stg = consts.tile([128, 3, HID], F32, tag="w2stg")
            for kb in range(3):
                nc.sync.dma_start(out=w2stg[:, kb, :],
                                  in_=dram["mlp_w2"][kb * 128:(kb + 1) * 128, :])
            w2b = consts.tile([128, 3, HID], BF16, tag="w2b")
            for kb in range(3):
                rs2 = consts.tile([128, 1], F32, tag="rs2")
                nc.vector.reduce_sum(rs2, w2stg[:, kb, :], mybir.AxisListType.X)
                rm2 = consts.tile([128, 1], F32, tag="rm2")
                nc.vector.tensor_scalar_mul(rm2, rs2, INV_HID)
                nc.vector.tensor_scalar_sub(w2b[:, kb, :], w2stg[:, kb, :], rm2)

            # b2 centered, [1, HID]
            b2stg = consts.tile([1, HID], F32, tag="b2stg")
            nc.sync.dma_start(out=b2stg, in_=dram["mlp_b2"][None, :])
            b2s = consts.tile([1, 1], F32, tag="b2s")
            nc.vector.reduce_sum(b2s, b2stg, mybir.AxisListType.X)
            b2m = consts.tile([1, 1], F32, tag="b2m")
            nc.vector.tensor_scalar_mul(b2m, b2s, INV_HID)
            b2c = consts.tile([1, HID], BF16, tag="b2c")
            nc.vector.tensor_scalar_sub(b2c, b2stg, b2m)

            wob_stg = consts.tile([128, 3, CD], F32, tag="wostg")
            for kb in range(3):
                nc.sync.dma_start(out=wob_stg[:, kb, :],
                                  in_=dram["out_w"][kb * 128:(kb + 1) * 128, :])
            wob = consts.tile([128, 3, CD], BF16, tag="wob")
            nc.vector.tensor_copy(wob, wob_stg)
            bout_row = staged("boutr", [1, CD], False,
                              [((slice(0, 1), slice(0, CD)), dram["out_b"][None, :])])

            # ---- fields
            F_f = fields.tile([CA, NPIX + 2], BF16, tag="F_f")
            RW_f = fields.tile([CA, NPIX + 256], BF16, tag="RW_f")
            B_f = fields.tile([CA, NPIX], BF16, tag="B_f")
            s2dw = fields.tile([128, 128], F32, tag="s2dw")
            E2dw = fields.tile([128, 128], F32, tag="E2dw")
            nc.vector.memset(F_f[:, 0:1], 0.0)
            nc.vector.memset(F_f[:, NPIX + 1:NPIX + 2], 0.0)
            nc.vector.memset(RW_f[:, 0:128], 0.0)
            nc.vector.memset(RW_f[:, NPIX + 128:NPIX + 256], 0.0)
            Fc = F_f[:, 1:1 + NPIX]
            RWc = RW_f[:, 128:128 + NPIX]

            # ---- phase A: load, gate dot, E, u = E*feat_aug, transpose into Fc
            with tc.tile_pool(name="ps_t", bufs=3, space="PSUM") as ps_t:
                for g in range(NBLK // GRP):
                    pxs = []
                    for j in range(GRP):
                        b = g * GRP + j
                        px = pxp.tile([128, CA], F32, tag="px")
                        nc.sync.dma_start(out=px[:, 0:CD],
                                          in_=codes[b * 128:(b + 1) * 128, :])
                        nc.sync.dma_start(out=px[:, CD:C],
                                          in_=depth[b * 128:(b + 1) * 128, :])
                        nc.vector.memset(px[:, C:CA], 1.0)
                        scr = scrapp.tile([128, CA], F32, tag="gscr")
                        nc.vector.tensor_mul(scr, px, gw_rep)
                        nc.vector.reduce_sum(s2dw[:, b:b + 1], scr,
                                             mybir.AxisListType.X)
                        pxs.append(px)
                    nc.scalar.activation(
                        out=E2dw[:, g * GRP:(g + 1) * GRP],
                        in_=s2dw[:, g * GRP:(g + 1) * GRP], func=AF.Exp)
                    for j in range(GRP):
                        b = g * GRP + j
                        upx = pxp.tile([128, CA], BF16, tag="upx")
                        nc.scalar.activation(out=upx, in_=pxs[j], func=AF.Copy,
                                             scale=E2dw[:, b:b + 1])
                        tp = ps_t.tile([CA, 128], BF16, tag="tp")
                        nc.tensor.transpose(tp, upx, i128b)
                        nc.vector.tensor_copy(Fc[:, b * 128:(b + 1) * 128], tp)

            # ---- phase B: separable 3x3 box with reflect boundary -> B_f
            for c in range(NCHUNK):
                sl = slice(c * 512, (c + 1) * 512)
                t = scrapp.tile([CA, 512], BF16, tag="boxt")
                nc.vector.tensor_add(t, F_f[:, c * 512:c * 512 + 512],
                                     F_f[:, c * 512 + 2:c * 512 + 514])
                nc.vector.tensor_add(RWc[:, sl], t, Fc[:, sl])
            Fv = Fc.rearrange("p (h w) -> p h w", h=H)
            RWv = RWc.rearrange("p (h w) -> p h w", h=H)
            nc.vector.scalar_tensor_tensor(
                out=RWv[:, :, 0:1], in0=Fv[:, :, 1:2], scalar=2.0,
                in1=Fv[:, :, 0:1], op0=ALU.mult, op1=ALU.add)
            nc.vector.scalar_tensor_tensor(
                out=RWv[:, :, 127:128], in0=Fv[:, :, 126:127], scalar=2.0,
                in1=Fv[:, :, 127:128], op0=ALU.mult, op1=ALU.add)
            for c in range(NCHUNK):
                sl = slice(c * 512, (c + 1) * 512)
                t = scrapp.tile([CA, 512], BF16, tag="boxt")
                nc.vector.tensor_add(t, RW_f[:, c * 512:c * 512 + 512],
                                     RW_f[:, c * 512 + 256:c * 512 + 768])
                nc.vector.tensor_add(B_f[:, sl], t, RWc[:, sl])
            nc.vector.scalar_tensor_tensor(
                out=B_f[:, 0:128], in0=RWc[:, 128:256], scalar=2.0,
                in1=RWc[:, 0:128], op0=ALU.mult, op1=ALU.add)
            nc.vector.scalar_tensor_tensor(
                out=B_f[:, NPIX - 128:NPIX], in0=RWc[:, NPIX - 256:NPIX - 128],
                scalar=2.0, in1=RWc[:, NPIX - 128:NPIX], op0=ALU.mult, op1=ALU.add)

            # ---- phase C: channel-major MLPs
            with (
                tc.tile_pool(name="ps_y1", bufs=2, space="PSUM") as ps_y1,
                tc.tile_pool(name="ps_y2", bufs=2, space="PSUM") as ps_y2,
                tc.tile_pool(name="ps_m", bufs=2, space="PSUM") as ps_m,
                tc.tile_pool(name="ps_o", bufs=2, space="PSUM") as ps_o,
            ):
                for c in range(NCHUNK):
                    Bc = B_f[:, c * 512:(c + 1) * 512]

                    # var1' = B.T M1 B / HID  (quadratic form, [1,512] row)
                    t1 = ps_y1.tile([128, 512], F32, tag="y1")
                    nc.tensor.matmul(t1[0:CA, :], lhsT=m1, rhs=Bc,
                                     start=True, stop=True)
                    prod = prodp.tile([CA, 512], BF16, tag="prod")
                    nc.vector.tensor_mul(prod, t1[0:CA, :], Bc)
                    v1row = ps_m.tile([128, 512], F32, tag="m")
                    nc.tensor.matmul(v1row[0:1, :], lhsT=inv_col[0:CA, :],
                                     rhs=prod, start=True, stop=True)
                    std1row = rowsp.tile([1, 512], BF16, tag="std1")
                    nc.scalar.activation(out=std1row, in_=v1row[0:1, :],
                                         func=AF.Sqrt, bias=eps_t[0:1, :],
                                         scale=1.0)

                    # mm1 (channel-major) + x1 = relu(y1')
                    x1s = []
                    for m in range(3):
                        yps = ps_y1.tile([128, 512], F32, tag="y1")
                        nc.tensor.matmul(yps, lhsT=wc[:, m * 128:(m + 1) * 128],
                                         rhs=Bc, start=True, stop=True)
                        x1 = xsp.tile([128, 512], BF16, tag="x1")
                        if m == 0:
                            nc.vector.tensor_scalar_max(out=x1, in0=yps,
                                                        scalar1=0.0)
                        else:
                            nc.scalar.activation(out=x1, in_=yps, func=AF.Relu)
                        x1s.append(x1)

                    # mm2 + sq2 + x2 = relu(y2')
                    x2s = []
                    sqs = []
                    for m in range(3):
                        yps = ps_y2.tile([128, 512], F32, tag="y2")
                        for kb in range(3):
                            nc.tensor.matmul(
                                yps, lhsT=w2b[:, kb, m * 128:(m + 1) * 128],
                                rhs=x1s[kb], start=(kb == 0), stop=False)
                        nc.tensor.matmul(yps, lhsT=b2c[0:1, m * 128:(m + 1) * 128],
                                         rhs=std1row, start=False, stop=True)
                        sq = sqp.tile([128, 512], BF16, tag="sq")
                        nc.scalar.activation(out=sq, in_=yps, func=AF.Square,
                                             scale=SQRT_INV_HID)
                        x2 = xsp.tile([128, 512], BF16, tag="x2")
                        if m < 2:
                            nc.vector.tensor_scalar_max(out=x2, in0=yps,
                                                        scalar1=0.0)
                        else:
                            nc.scalar.activation(out=x2, in_=yps, func=AF.Relu)
                        x2s.append(x2)
                        sqs.append(sq)

                    v2row = ps_m.tile([128, 512], F32, tag="m")
                    for m in range(3):
                        nc.tensor.matmul(v2row[0:1, :], lhsT=ones_col,
                                         rhs=sqs[m], start=(m == 0),
                                         stop=(m == 2))
                    std2row = rowsp.tile([1, 512], BF16, tag="std2")
                    nc.scalar.activation(out=std2row, in_=v2row[0:1, :],
                                         func=AF.Sqrt, bias=eps_t[0:1, :],
                                         scale=1.0)

                    # rstd2 transposed to pixel-major via rank-1 matmuls
                    stdT = ps_m.tile([128, 512], F32, tag="m")
                    for i in range(4):
                        nc.tensor.matmul(
                            stdT[:, i:i + 1],
                            lhsT=std2row[0:1, i * 128:(i + 1) * 128],
                            rhs=ones_col[0:1, :], start=True, stop=True)
                    stdTc = rowsp.tile([128, 4], F32, tag="stdTc")
                    nc.vector.tensor_copy(stdTc, stdT[:, 0:4])
                    rstdT = rowsp.tile([128, 4], F32, tag="rstdT")
                    nc.vector.reciprocal(rstdT, stdTc)

                    # mm3 pixel-major + finalize (4 blocks packed in one bank)
                    o_tile = ps_o.tile([128, 512], F32, tag="o")
                    for blk in range(4):
                        b = c * 4 + blk
                        ops = o_tile[:, blk * 128:blk * 128 + CD]
                        for kb in range(3):
                            nc.tensor.matmul(
                                ops, lhsT=x2s[kb][:, blk * 128:(blk + 1) * 128],
                                rhs=wob[:, kb, :], start=(kb == 0), stop=False)
                        nc.tensor.matmul(
                            ops, lhsT=std2row[0:1, blk * 128:(blk + 1) * 128],
                            rhs=bout_row, start=False, stop=True)
                        ot = outp.tile([128, CD], F32, tag="ot")
                        nc.scalar.activation(out=ot, in_=ops, func=AF.Copy,
                                             scale=rstdT[:, blk:blk + 1])
                        cb = outp.tile([128, CD], F32, tag="cb")
                        nc.sync.dma_start(out=cb,
                                          in_=codes[b * 128:(b + 1) * 128, :])
                        fin = outp.tile([128, CD], F32, tag="fin")
                        nc.vector.tensor_add(fin, ot, cb)
                        nc.sync.dma_start(out=out[b * 128:(b + 1) * 128, :],
                                          in_=fin)

    nc.compile()
    return nc


_CACHED = {}


def kernel(**inputs) -> np.ndarray:
    codes = np.ascontiguousarray(np.asarray(inputs["codes"], dtype=np.float32))
    depth = np.ascontiguousarray(np.asarray(inputs["depth"], dtype=np.float32))
    B = codes.shape[0]
    assert codes.shape == (B, NPIX, CD) and depth.shape == (B, NPIX, DD)
    assert int(inputs["ph"]) == H and int(inputs["pw"]) == W

    ln_identity = (
        np.allclose(np.asarray(inputs["ln1_g"]), 1.0)
        and np.allclose(np.asarray(inputs["ln1_b"]), 0.0)
        and np.allclose(np.asarray(inputs["ln2_g"]), 1.0)
        and np.allclose(np.asarray(inputs["ln2_b"]), 0.0)
    )
    key = not ln_identity
    if key not in _CACHED:
        if ln_identity:
            _CACHED[key] = build_kernel_v2()
        else:
            _CACHED[key] = build_kernel(apply_ln_affine=True)
    nc = _CACHED[key]

    weights = {
        k: np.ascontiguousarray(np.asarray(inputs[k], dtype=np.float32))
        for k in ["attn_proj_w", "attn_proj_b", "attn_gate_w", "mlp_w1",
                  "mlp_b1", "ln1_g", "ln1_b", "mlp_w2", "mlp_b2", "ln2_g",
                  "ln2_b", "out_w", "out_b"]
    }
    weights["attn_gate_w"] = weights["attn_gate_w"].reshape(C, 1)

    n_cores = 8
    in_maps = []
    for core in range(n_cores):
        b = core % B
        in_maps.append({"codes": codes[b], "depth": depth[b], **weights})

    res = run_bass_kernel_spmd(nc, in_maps, core_ids=list(range(n_cores)))
    out = np.stack([res.results[core % n_cores]["out"] for core in range(B)], axis=0)
    return out.astype(np.float32)


if __name__ == "__main__":
    import reference

    inputs = reference.setup_inputs()
    expected = np.asarray(reference.reference(**inputs))
    actual = kernel(**{kk: np.asarray(v) if hasattr(v, "shape") else v
                       for kk, v in inputs.items()})
    err = np.linalg.norm(actual - expected) / np.linalg.norm(expected)
    print("Relative error:", err)



# revision 18
# speedup vs baseline: 1.3283x; 1.3283x over previous
"""Trainium2 Bass kernel for DepthAdapterWindowAttn.

Math (per batch image, H=W=128, C=106 feat channels):
  feat = concat(codes, depth)                              # (N, 106)
  s    = feat @ gate_w            (gate bias dropped: softmax-invariant)
  E    = exp(s)                   (no max-subtract needed: |s| ~ N(0,1))
  p    = feat @ Wproj + b         (proj of a shifted window == shift of proj)
  F    = [E*p ; E]                # 107 channels
  G    = box3x3_reflect(F)        # separable: W-pass then H-pass
  attended = G[0:106] / G[106]    # softmax-weighted window sum
  y1 = attended @ W1 + b1 ; x1 = relu(LN(y1))
  y2 = x1 @ W2 + b2       ; x2 = relu(LN(y2))
  out = codes + x2 @ Wout + bout

Key trick: LayerNorm is invariant to a positive per-pixel scale, so the
division by Z = G[106] cancels inside LN1:  mm1 consumes the *unnormalized*
box output G directly, with w1_aug row 106 (= b1) multiplied by the Z row,
which scales the bias by exactly the right factor.  The softmax denominator
is never divided out explicitly.

Sharding: data-parallel over batch B=8, one image per NeuronCore.
"""

import numpy as np

import concourse.bacc as bacc
import concourse.bass as bass
import concourse.mybir as mybir
import concourse.tile as tile
from concourse.bass_utils import run_bass_kernel_spmd
from concourse.masks import make_identity

F32 = mybir.dt.float32
BF16 = mybir.dt.bfloat16
AF = mybir.ActivationFunctionType
ALU = mybir.AluOpType

H = 128
W = 128
NPIX = H * W            # 16384
CD = 90                 # code dim
DD = 16                 # depth dim
C = CD + DD             # 106
CA = C + 1              # 107 (augmented with ones/E row)
HID = 384
EPS = 1e-5
NCHUNK = NPIX // 512    # 32
NBLK = NPIX // 128      # 128
GRP = 16                # LN stat batching group


def _consts(nc, tc, consts, dram, apply_ln_affine):
    """Load/cast all weights into SBUF bf16 tiles."""
    i128b = consts.tile([128, 128], BF16, tag="i128b")
    make_identity(nc, i128b)

    ones1 = consts.tile([1, 128], BF16, tag="ones1")
    nc.vector.memset(ones1, 1.0)

    eps_t = consts.tile([128, 1], F32, tag="eps_t")
    nc.vector.memset(eps_t, EPS)

    def staged(name, shape_dst, fill_zero, loads, dtype=BF16):
        stg = consts.tile(shape_dst, F32, tag=f"stg_{name}")
        if fill_zero:
            nc.vector.memset(stg, 0.0)
        for dst_sl, src_ap in loads:
            nc.sync.dma_start(out=stg[dst_sl], in_=src_ap)
        t = consts.tile(shape_dst, dtype, tag=name)
        nc.vector.tensor_copy(t, stg)
        return t

    def bcast_ap(handle, n):
        ap = handle[:]
        return bass.AP(tensor=ap.tensor, offset=ap.offset, ap=[[0, 128], [1, n]])

    k = {}
    # Wpg_aug[kin, mout]: kin 106 = ones row, mout 106 = E pass-through
    wpg = staged(
        "wpg", [CA, CA], True,
        [((slice(0, C), slice(0, C)), dram["attn_proj_w"][:, :]),
         ((slice(C, CA), slice(0, C)), dram["attn_proj_b"][None, :])])
    # E pass-through column: wpg[:, 106] = e_106, taken from the identity
    # (avoids a single-partition write at partition 106, which BIR rejects)
    nc.vector.tensor_copy(wpg[0:CA, C:C + 1], i128b[0:CA, C:C + 1])
    k["wpg"] = wpg

    k["gw_rep"] = staged(
        "gw_rep", [128, CA], True,
        [((slice(0, 128), slice(0, C)), bcast_ap(dram["attn_gate_w"], C))],
        dtype=F32)

    k["w1a"] = staged(
        "w1a", [CA, HID], False,
        [((slice(0, C), slice(0, HID)), dram["mlp_w1"][:, :]),
         ((slice(C, CA), slice(0, HID)), dram["mlp_b1"][None, :])])

    w2b_stg = consts.tile([128, 3, HID], F32, tag="w2stg")
    for kb in range(3):
        nc.sync.dma_start(out=w2b_stg[:, kb, :],
                          in_=dram["mlp_w2"][kb * 128:(kb + 1) * 128, :])
    w2b = consts.tile([128, 3, HID], BF16, tag="w2b")
    nc.vector.tensor_copy(w2b, w2b_stg)
    k["w2b"] = w2b

    wob_stg = consts.tile([128, 3, CD], F32, tag="wostg")
    for kb in range(3):
        nc.sync.dma_start(out=wob_stg[:, kb, :],
                          in_=dram["out_w"][kb * 128:(kb + 1) * 128, :])
    wob = consts.tile([128, 3, CD], BF16, tag="wob")
    nc.vector.tensor_copy(wob, wob_stg)
    k["wob"] = wob

    k["b2_row"] = staged("b2r", [1, HID], False,
                         [((slice(0, 1), slice(0, HID)), dram["mlp_b2"][None, :])])
    k["ob_row"] = staged("obr", [1, CD], False,
                         [((slice(0, 1), slice(0, CD)), dram["out_b"][None, :])])

    if apply_ln_affine:
        k["g1_rep"] = staged("g1r", [128, HID], False,
                             [((slice(0, 128), slice(0, HID)), bcast_ap(dram["ln1_g"], HID))])
        k["b1_rep"] = staged("b1r", [128, HID], False,
                             [((slice(0, 128), slice(0, HID)), bcast_ap(dram["ln1_b"], HID))])
        k["g2_rep"] = staged("g2r", [128, HID], False,
                             [((slice(0, 128), slice(0, HID)), bcast_ap(dram["ln2_g"], HID))])
        k["b2l_rep"] = staged("b2lr", [128, HID], False,
                              [((slice(0, 128), slice(0, HID)), bcast_ap(dram["ln2_b"], HID))])
    k["i128b"] = i128b
    k["ones1"] = ones1
    k["eps_t"] = eps_t
    return k


def build_kernel(apply_ln_affine: bool) -> bass.Bass:
    nc = bacc.Bacc("TRN2", target_bir_lowering=False, num_devices=8)

    dram = {}
    dram["codes"] = nc.declare_dram_parameter("codes", [NPIX, CD], F32, isOutput=False)
    dram["depth"] = nc.declare_dram_parameter("depth", [NPIX, DD], F32, isOutput=False)
    for name, shape in [
        ("attn_proj_w", [C, C]), ("attn_proj_b", [C]), ("attn_gate_w", [C, 1]),
        ("mlp_w1", [C, HID]), ("mlp_b1", [HID]), ("ln1_g", [HID]), ("ln1_b", [HID]),
        ("mlp_w2", [HID, HID]), ("mlp_b2", [HID]), ("ln2_g", [HID]), ("ln2_b", [HID]),
        ("out_w", [HID, CD]), ("out_b", [CD]),
    ]:
        dram[name] = nc.declare_dram_parameter(name, shape, F32, isOutput=False)
    out = nc.declare_dram_parameter("out", [NPIX, CD], F32, isOutput=True)
    codes = dram["codes"]
    depth = dram["depth"]

    with tile.TileContext(nc) as tc:
        with (
            tc.tile_pool(name="consts", bufs=1) as consts,
            tc.tile_pool(name="fields", bufs=1) as fields,
            tc.tile_pool(name="px", bufs=GRP + 2) as pxp,
            tc.tile_pool(name="uchunk", bufs=3) as uchp,
            tc.tile_pool(name="scrap", bufs=3) as scrapp,
            tc.tile_pool(name="ysb", bufs=GRP + 2) as ysbp,
            tc.tile_pool(name="xn", bufs=3) as xnp,
            tc.tile_pool(name="xt", bufs=3) as xtp,
            tc.tile_pool(name="stats", bufs=2) as statsp,
            tc.tile_pool(name="outp", bufs=4) as outp,
        ):
            k = _consts(nc, tc, consts, dram, apply_ln_affine)
            i128b, ones1, eps_t = k["i128b"], k["ones1"], k["eps_t"]

            # x = h*128 + w pixel flattening; free-dim pads for box shifts
            F_f = fields.tile([CA, NPIX + 2], BF16, tag="F_f")    # center off 1
            RW_f = fields.tile([CA, NPIX + 256], BF16, tag="RW_f")  # center off 128
            G_f = fields.tile([CA, NPIX], BF16, tag="G_f")
            s2dw = fields.tile([128, 128], F32, tag="s2dw")       # s[w, h]
            E2dw = fields.tile([128, 128], F32, tag="E2dw")
            nc.vector.memset(F_f[:, 0:1], 0.0)
            nc.vector.memset(F_f[:, NPIX + 1:NPIX + 2], 0.0)
            nc.vector.memset(RW_f[:, 0:128], 0.0)
            nc.vector.memset(RW_f[:, NPIX + 128:NPIX + 256], 0.0)
            Fc = F_f[:, 1:1 + NPIX]
            RWc = RW_f[:, 128:128 + NPIX]

            # ---- phase A: load, gate dot, E, u = E*feat, transpose-in, proj
            with (
                tc.tile_pool(name="ps_t", bufs=3, space="PSUM") as ps_t,
                tc.tile_pool(name="ps_f", bufs=2, space="PSUM") as ps_f,
            ):
                for g in range(NBLK // GRP):
                    pxs = []
                    for j in range(GRP):
                        b = g * GRP + j
                        px = pxp.tile([128, CA], F32, tag="px")
                        nc.sync.dma_start(out=px[:, 0:CD],
                                          in_=codes[b * 128:(b + 1) * 128, :])
                        nc.sync.dma_start(out=px[:, CD:C],
                                          in_=depth[b * 128:(b + 1) * 128, :])
                        nc.vector.memset(px[:, C:CA], 1.0)
                        scr = scrapp.tile([128, CA], F32, tag="sscr")
                        nc.vector.tensor_mul(scr, px, k["gw_rep"])
                        nc.vector.reduce_sum(s2dw[:, b:b + 1], scr,
                                             mybir.AxisListType.X)
                        pxs.append(px)
                    nc.scalar.activation(
                        out=E2dw[:, g * GRP:(g + 1) * GRP],
                        in_=s2dw[:, g * GRP:(g + 1) * GRP], func=AF.Exp)
                    for j in range(GRP):
                        b = g * GRP + j
                        upx = pxp.tile([128, CA], BF16, tag="upx")
                        nc.scalar.activation(out=upx, in_=pxs[j], func=AF.Copy,
                                             scale=E2dw[:, b:b + 1])
                        tp = ps_t.tile([CA, 128], BF16, tag="tp")
                        nc.tensor.transpose(tp, upx, i128b)
                        if b % 4 == 0:
                            uch = uchp.tile([CA, 512], BF16, tag="uc")
                        if b % 2 == 0:
                            nc.vector.tensor_copy(
                                uch[:, (b % 4) * 128:(b % 4 + 1) * 128], tp)
                        else:
                            nc.scalar.copy(
                                uch[:, (b % 4) * 128:(b % 4 + 1) * 128], tp)
                        if b % 4 == 3:
                            c = b // 4
                            fps = ps_f.tile([CA, 512], F32, tag="fps")
                            nc.tensor.matmul(fps, lhsT=k["wpg"], rhs=uch,
                                             start=True, stop=True)
                            if c % 2 == 0:
                                nc.vector.tensor_copy(Fc[:, c * 512:(c + 1) * 512], fps)
                            else:
                                nc.scalar.copy(Fc[:, c * 512:(c + 1) * 512], fps)

            # ---- phase B: separable 3x3 box with reflect boundary
            for c in range(NCHUNK):
                sl = slice(c * 512, (c + 1) * 512)
                t = scrapp.tile([CA, 512], BF16, tag="boxt")
                nc.vector.tensor_add(t, F_f[:, c * 512:c * 512 + 512],
                                     F_f[:, c * 512 + 2:c * 512 + 514])
                nc.vector.tensor_add(RWc[:, sl], t, Fc[:, sl])
            Fv = Fc.rearrange("p (h w) -> p h w", h=H)
            RWv = RWc.rearrange("p (h w) -> p h w", h=H)
            nc.vector.scalar_tensor_tensor(
                out=RWv[:, :, 0:1], in0=Fv[:, :, 1:2], scalar=2.0,
                in1=Fv[:, :, 0:1], op0=ALU.mult, op1=ALU.add)
            nc.vector.scalar_tensor_tensor(
                out=RWv[:, :, 127:128], in0=Fv[:, :, 126:127], scalar=2.0,
                in1=Fv[:, :, 127:128], op0=ALU.mult, op1=ALU.add)
            for c in range(NCHUNK):
                sl = slice(c * 512, (c + 1) * 512)
                t = scrapp.tile([CA, 512], BF16, tag="boxt")
                nc.vector.tensor_add(t, RW_f[:, c * 512:c * 512 + 512],
                                     RW_f[:, c * 512 + 256:c * 512 + 768])
                nc.vector.tensor_add(G_f[:, sl], t, RWc[:, sl])
            nc.vector.scalar_tensor_tensor(
                out=G_f[:, 0:128], in0=RWc[:, 128:256], scalar=2.0,
                in1=RWc[:, 0:128], op0=ALU.mult, op1=ALU.add)
            nc.vector.scalar_tensor_tensor(
                out=G_f[:, NPIX - 128:NPIX], in0=RWc[:, NPIX - 256:NPIX - 128],
                scalar=2.0, in1=RWc[:, NPIX - 128:NPIX], op0=ALU.mult, op1=ALU.add)

            # ---- phase C: MLPs + LNs + residual
            def ln_rstd_batched(mv):
                """mv [128, GRP, 2] (mean, var) -> rstd [128, GRP]."""
                sd = statsp.tile([128, GRP], F32, tag="sd")
                nc.scalar.activation(out=sd, in_=mv[:, :, 1], func=AF.Sqrt,
                                     bias=eps_t, scale=1.0)
                rstd = statsp.tile([128, GRP], F32, tag="rstd")
                nc.vector.reciprocal(rstd, sd)
                return rstd

            def neg_mu_rstd(mv, rstd):
                """[128, GRP] tile of -mean * rstd (fused LN bias)."""
                nm = statsp.tile([128, GRP], F32, tag="nm")
                nc.vector.scalar_tensor_tensor(
                    out=nm, in0=mv[:, :, 0], scalar=-1.0, in1=rstd,
                    op0=ALU.mult, op1=ALU.mult)
                return nm

            def apply_ln_relu(xn, y_sb, mv, nm, rstd, j, gamma_rep, beta_rep):
                """x = relu(LN(y)): one fused scalar-engine op when the LN
                affine is identity (relu commutes with the positive rstd
                scale: Relu(y*rstd - mu*rstd) == relu(LN(y)))."""
                if gamma_rep is None:
                    nc.scalar.activation(
                        out=xn, in_=y_sb, func=AF.Relu,
                        bias=nm[:, j:j + 1], scale=rstd[:, j:j + 1])
                else:
                    nc.vector.tensor_scalar(
                        out=xn, in0=y_sb, scalar1=mv[:, j, 0:1],
                        scalar2=rstd[:, j:j + 1],
                        op0=ALU.subtract, op1=ALU.mult)
                    nc.vector.tensor_mul(xn, xn, gamma_rep)
                    nc.vector.tensor_add(xn, xn, beta_rep)
                    nc.scalar.activation(out=xn, in_=xn, func=AF.Relu)

            with (
                tc.tile_pool(name="ps_y", bufs=2, space="PSUM") as ps_y,
                tc.tile_pool(name="ps_xt", bufs=2, space="PSUM") as ps_xt,
                tc.tile_pool(name="ps_o", bufs=2, space="PSUM") as ps_o,
            ):
                def transpose_copy(xn):
                    """xn (already relu'd) -> 3 transposed [128,128] tiles."""
                    xt = xtp.tile([128, 3, 128], BF16, tag="xt")
                    tps = ps_xt.tile([128, 3, 128], BF16, tag="tps")
                    for kb in range(3):
                        nc.tensor.transpose(tps[:, kb, :],
                                            xn[:, kb * 128:(kb + 1) * 128], i128b)
                        if kb == 1:
                            nc.scalar.copy(xt[:, kb, :], tps[:, kb, :])
                        else:
                            nc.vector.tensor_copy(xt[:, kb, :], tps[:, kb, :])
                    return xt

                for g in range(NBLK // GRP):
                    mv1 = statsp.tile([128, GRP, 2], F32, tag="mv1")
                    y1s = []
                    for j in range(GRP):
                        b = g * GRP + j
                        yps = ps_y.tile([128, HID], F32, tag="yps")
                        nc.tensor.matmul(yps, lhsT=G_f[:, b * 128:(b + 1) * 128],
                                         rhs=k["w1a"], start=True, stop=True)
                        y_sb = ysbp.tile([128, HID], BF16, tag="y1sb")
                        nc.scalar.copy(y_sb, yps)
                        st = scrapp.tile([128, 6], F32, tag="st")
                        nc.vector.bn_stats(out=st, in_=yps)
                        nc.vector.bn_aggr(out=mv1[:, j, :], in_=st)
                        y1s.append(y_sb)
                    rstd1 = ln_rstd_batched(mv1)
                    nm1 = neg_mu_rstd(mv1, rstd1)

                    mv2 = statsp.tile([128, GRP, 2], F32, tag="mv2")
                    y2s = []
                    for j in range(GRP):
                        b = g * GRP + j
                        xn = xnp.tile([128, HID], BF16, tag="x1n")
                        apply_ln_relu(xn, y1s[j], mv1, nm1, rstd1, j,
                                      k.get("g1_rep"), k.get("b1_rep"))
                        xt = transpose_copy(xn)
                        yps = ps_y.tile([128, HID], F32, tag="yps")
                        for kb in range(3):
                            nc.tensor.matmul(yps, lhsT=xt[:, kb, :],
                                             rhs=k["w2b"][:, kb, :],
                                             start=(kb == 0), stop=False)
                        nc.tensor.matmul(yps, lhsT=ones1, rhs=k["b2_row"],
                                         start=False, stop=True)
                        y_sb = ysbp.tile([128, HID], BF16, tag="y2sb")
                        nc.scalar.copy(y_sb, yps)
                        st = scrapp.tile([128, 6], F32, tag="st")
                        nc.vector.bn_stats(out=st, in_=yps)
                        nc.vector.bn_aggr(out=mv2[:, j, :], in_=st)
                        y2s.append(y_sb)
                    rstd2 = ln_rstd_batched(mv2)
                    nm2 = neg_mu_rstd(mv2, rstd2)

                    for j in range(GRP):
                        b = g * GRP + j
                        xn = xnp.tile([128, HID], BF16, tag="x2n")
                        apply_ln_relu(xn, y2s[j], mv2, nm2, rstd2, j,
                                      k.get("g2_rep"), k.get("b2l_rep"))
                        xt = transpose_copy(xn)
                        ops = ps_o.tile([128, CD], F32, tag="ops")
                        for kb in range(3):
                            nc.tensor.matmul(ops, lhsT=xt[:, kb, :],
                                             rhs=k["wob"][:, kb, :],
                                             start=(kb == 0), stop=False)
                        nc.tensor.matmul(ops, lhsT=ones1, rhs=k["ob_row"],
                                         start=False, stop=True)
                        cb = outp.tile([128, CD], F32, tag="cb")
                        nc.sync.dma_start(out=cb,
                                          in_=codes[b * 128:(b + 1) * 128, :])
                        ot = outp.tile([128, CD], F32, tag="ot")
                        nc.vector.tensor_add(ot, ops, cb)
                        nc.sync.dma_start(out=out[b * 128:(b + 1) * 128, :], in_=ot)

    nc.compile()
    return nc


def build_kernel_v2() -> bass.Bass:
    """Channel-major phase C: no per-block transposes, matmul-based LN stats.

    Tricks on top of v1:
      - Wpg folds into W1:  y1 = (Wpg @ W1aug).T @ box(u), so the proj matmul
        and the F field disappear; the box filter runs directly on
        u = [E*feat ; E].
      - Column-centered weights: subtracting each row's mean over the output
        dim from Wc / W2 / b2 makes mean(y) == 0 exactly, killing the LN mean
        computation (LN is then y * rstd).
      - Channel-major mm1/mm2: LN variance via sum of squares over the
        partition dim = ones-vector matmuls; var1 via the quadratic form
        B.T (Wc' Wc'.T) B (one [107x107] matmul + one dot pass).
      - Deferred normalization: x1 = relu(y1') unscaled (LN2 is invariant to
        positive per-pixel scale); b2 rides a rank-1 matmul scaled by std1;
        the exact rstd2 scale is applied pixel-major on the 90-dim output.
    """
    nc = bacc.Bacc("TRN2", target_bir_lowering=False, num_devices=8)

    dram = {}
    dram["codes"] = nc.declare_dram_parameter("codes", [NPIX, CD], F32, isOutput=False)
    dram["depth"] = nc.declare_dram_parameter("depth", [NPIX, DD], F32, isOutput=False)
    for name, shape in [
        ("attn_proj_w", [C, C]), ("attn_proj_b", [C]), ("attn_gate_w", [C, 1]),
        ("mlp_w1", [C, HID]), ("mlp_b1", [HID]), ("ln1_g", [HID]), ("ln1_b", [HID]),
        ("mlp_w2", [HID, HID]), ("mlp_b2", [HID]), ("ln2_g", [HID]), ("ln2_b", [HID]),
        ("out_w", [HID, CD]), ("out_b", [CD]),
    ]:
        dram[name] = nc.declare_dram_parameter(name, shape, F32, isOutput=False)
    out = nc.declare_dram_parameter("out", [NPIX, CD], F32, isOutput=True)
    codes = dram["codes"]
    depth = dram["depth"]
    INV_HID = 1.0 / HID
    SQRT_INV_HID = float(np.sqrt(1.0 / HID))

    with tile.TileContext(nc) as tc:
        with (
            tc.tile_pool(name="consts", bufs=1) as consts,
            tc.tile_pool(name="fields", bufs=1) as fields,
            tc.tile_pool(name="px", bufs=2) as pxp,
            tc.tile_pool(name="upx", bufs=6) as upxp,
            tc.tile_pool(name="scrap", bufs=4) as scrapp,
            tc.tile_pool(name="xs", bufs=9) as xsp,
            tc.tile_pool(name="sq", bufs=6) as sqp,
            tc.tile_pool(name="prod", bufs=3) as prodp,
            tc.tile_pool(name="rows", bufs=4) as rowsp,
            tc.tile_pool(name="outp", bufs=8) as outp,
        ):
            # ---- constants / weight prep
            i128b = consts.tile([128, 128], BF16, tag="i128b")
            make_identity(nc, i128b)
            ones_col = consts.tile([128, 1], BF16, tag="ones_col")
            nc.vector.memset(ones_col, 1.0)
            inv_col = consts.tile([128, 1], BF16, tag="inv_col")
            nc.vector.memset(inv_col, INV_HID)
            eps_t = consts.tile([128, 1], F32, tag="eps_t")
            nc.vector.memset(eps_t, EPS)

            def staged(name, shape_dst, fill_zero, loads, dtype=BF16):
                stg = consts.tile(shape_dst, F32, tag=f"stg_{name}")
                if fill_zero:
                    nc.vector.memset(stg, 0.0)
                for dst_sl, src_ap in loads:
                    nc.sync.dma_start(out=stg[dst_sl], in_=src_ap)
                t = consts.tile(shape_dst, dtype, tag=name)
                nc.vector.tensor_copy(t, stg)
                return t

            def bcast_ap(handle, n):
                ap = handle[:]
                return bass.AP(tensor=ap.tensor, offset=ap.offset,
                               ap=[[0, 128], [1, n]])

            wpg = staged(
                "wpg", [CA, CA], True,
                [((slice(0, C), slice(0, C)), dram["attn_proj_w"][:, :]),
                 ((slice(C, CA), slice(0, C)), dram["attn_proj_b"][None, :])])
            nc.vector.tensor_copy(wpg[0:CA, C:C + 1], i128b[0:CA, C:C + 1])

            gw_rep = staged(
                "gw_rep", [128, CA], True,
                [((slice(0, 128), slice(0, C)), bcast_ap(dram["attn_gate_w"], C))],
                dtype=F32)

            w1a = staged(
                "w1a", [CA, HID], False,
                [((slice(0, C), slice(0, HID)), dram["mlp_w1"][:, :]),
                 ((slice(C, CA), slice(0, HID)), dram["mlp_b1"][None, :])])

            with tc.tile_pool(name="ps_prep", bufs=2, space="PSUM") as ps_prep:
                # Wc = rowcenter(Wpg @ W1aug)  [CA, HID]
                tp_ps = ps_prep.tile([128, 128], BF16, tag="prepb")
                nc.tensor.transpose(tp_ps[0:CA, 0:CA], wpg, i128b[0:CA, 0:CA])
                wpgT = consts.tile([CA, CA], BF16, tag="wpgT")
                nc.vector.tensor_copy(wpgT, tp_ps[0:CA, 0:CA])
                cb_ps = ps_prep.tile([128, 512], F32, tag="prepf")
                nc.tensor.matmul(cb_ps[0:CA, 0:HID], lhsT=wpgT, rhs=w1a,
                                 start=True, stop=True)
                rs = consts.tile([CA, 1], F32, tag="rs")
                nc.vector.reduce_sum(rs, cb_ps[0:CA, 0:HID], mybir.AxisListType.X)
                rm = consts.tile([CA, 1], F32, tag="rm")
                nc.vector.tensor_scalar_mul(rm, rs, INV_HID)
                wc = consts.tile([CA, HID], BF16, tag="wc")
                nc.vector.tensor_scalar_sub(wc, cb_ps[0:CA, 0:HID], rm)

                # M1 = Wc @ Wc.T  [CA, CA]
                wcT = consts.tile([128, 3, CA], BF16, tag="wcT")
                for kb in range(3):
                    t_ps = ps_prep.tile([128, 128], BF16, tag="prepb")
                    nc.tensor.transpose(t_ps[0:128, 0:CA],
                                        wc[:, kb * 128:(kb + 1) * 128],
                                        i128b[0:CA, 0:CA])
                    nc.vector.tensor_copy(wcT[:, kb, :], t_ps[0:128, 0:CA])
                m1_ps = ps_prep.tile([128, 512], F32, tag="prepf")
                for kb in range(3):
                    nc.tensor.matmul(m1_ps[0:CA, 0:CA], lhsT=wcT[:, kb, :],
                                     rhs=wcT[:, kb, :], start=(kb == 0),
                                     stop=(kb == 2))
                m1 = consts.tile([CA, CA], BF16, tag="m1")
                nc.vector.tensor_copy(m1, m1_ps[0:CA, 0:CA])

            # W2 row-centered over output dim, [128, 3, HID]
            w2stg = consts.tile([128, 3, HID], F32, tag="w2stg")
            for kb in range(3):
                nc.sync.dma_start(out=w2stg[:, kb, :],
                                  in_=dram["mlp_w2"][kb * 128:(kb + 1) * 128, :])
            w2b = consts.tile([128, 3, HID], BF16, tag="w2b")
            for kb in range(3):
                rs2 = consts.tile([128, 1], F32, tag="rs2")
                nc.vector.reduce_sum(rs2, w2stg[:, kb, :], mybir.AxisListType.X)
                rm2 = consts.tile([128, 1], F32, tag="rm2")
                nc.vector.tensor_scalar_mul(rm2, rs2, INV_HID)
                nc.vector.tensor_scalar_sub(w2b[:, kb, :], w2stg[:, kb, :], rm2)

            # b2 centered, [1, HID]
            b2stg = consts.tile([1, HID], F32, tag="b2stg")
            nc.sync.dma_start(out=b2stg, in_=dram["mlp_b2"][None, :])
            b2s = consts.tile([1, 1], F32, tag="b2s")
            nc.vector.reduce_sum(b2s, b2stg, mybir.AxisListType.X)
            b2m = consts.tile([1, 1], F32, tag="b2m")
            nc.vector.tensor_scalar_mul(b2m, b2s, INV_HID)
            b2c = consts.tile([1, HID], BF16, tag="b2c")
            nc.vector.tensor_scalar_sub(b2c, b2stg, b2m)

            wob_stg = consts.tile([128, 3, CD], F32, tag="wostg")
            for kb in range(3):
                nc.sync.dma_start(out=wob_stg[:, kb, :],
                                  in_=dram["out_w"][kb * 128:(kb + 1) * 128, :])
            wob = consts.tile([128, 3, CD], BF16, tag="wob")
            nc.vector.tensor_copy(wob, wob_stg)
            bout_row = staged("boutr", [1, CD], False,
                              [((slice(0, 1), slice(0, CD)), dram["out_b"][None, :])])

            # ---- fields
            F_f = fields.tile([CA, NPIX + 2], BF16, tag="F_f")
            RW_f = fields.tile([CA, NPIX + 256], BF16, tag="RW_f")
            B_f = fields.tile([CA, NPIX], BF16, tag="B_f")
            s2dw = fields.tile([128, 128], F32, tag="s2dw")
            E2dw = fields.tile([128, 128], F32, tag="E2dw")
            nc.vector.memset(F_f[:, 0:1], 0.0)
            nc.vector.memset(F_f[:, NPIX + 1:NPIX + 2], 0.0)
            nc.vector.memset(RW_f[:, 0:128], 0.0)
            nc.vector.memset(RW_f[:, NPIX + 128:NPIX + 256], 0.0)
            Fc = F_f[:, 1:1 + NPIX]
            RWc = RW_f[:, 128:128 + NPIX]

            # ---- phase A: load, gate dot, E, u = E*feat_aug, transpose into Fc
            def dram_grp(handle, row0, nblk, ncol):
                """3D AP over DRAM rows [row0, row0+nblk*128) as [128, nblk, ncol]."""
                ap = handle[:]
                return bass.AP(tensor=ap.tensor, offset=ap.offset + row0 * ncol,
                               ap=[[ncol, 128], [128 * ncol, nblk], [1, ncol]])

            with tc.tile_pool(name="ps_t", bufs=3, space="PSUM") as ps_t:
                for g in range(NBLK // GRP):
                    pxg = pxp.tile([128, GRP, CA], F32, tag="px")
                    nc.sync.dma_start(out=pxg[:, :, 0:CD],
                                      in_=dram_grp(codes, g * GRP * 128, GRP, CD))
                    nc.sync.dma_start(out=pxg[:, :, CD:C],
                                      in_=dram_grp(depth, g * GRP * 128, GRP, DD))
                    nc.vector.memset(pxg[:, :, C:CA], 1.0)
                    for j in range(GRP):
                        b = g * GRP + j
                        scr = scrapp.tile([128, CA], F32, tag="gscr")
                        nc.vector.tensor_mul(scr, pxg[:, j, :], gw_rep)
                        nc.vector.reduce_sum(s2dw[:, b:b + 1], scr,
                                             mybir.AxisListType.X)
                    nc.scalar.activation(
                        out=E2dw[:, g * GRP:(g + 1) * GRP],
                        in_=s2dw[:, g * GRP:(g + 1) * GRP], func=AF.Exp)
                    for j in range(GRP):
                        b = g * GRP + j
                        upx = upxp.tile([128, CA], BF16, tag="upx")
                        nc.scalar.activation(out=upx, in_=pxg[:, j, :],
                                             func=AF.Copy,
                                             scale=E2dw[:, b:b + 1])
                        tp = ps_t.tile([CA, 128], BF16, tag="tp")
                        nc.tensor.transpose(tp, upx, i128b)
                        nc.vector.tensor_copy(Fc[:, b * 128:(b + 1) * 128], tp)

            # ---- phase B: separable 3x3 box with reflect boundary -> B_f
            for c in range(NCHUNK):
                sl = slice(c * 512, (c + 1) * 512)
                t = scrapp.tile([CA, 512], BF16, tag="boxt")
                nc.vector.tensor_add(t, F_f[:, c * 512:c * 512 + 512],
                                     F_f[:, c * 512 + 2:c * 512 + 514])
                nc.vector.tensor_add(RWc[:, sl], t, Fc[:, sl])
            Fv = Fc.rearrange("p (h w) -> p h w", h=H)
            RWv = RWc.rearrange("p (h w) -> p h w", h=H)
            nc.vector.scalar_tensor_tensor(
                out=RWv[:, :, 0:1], in0=Fv[:, :, 1:2], scalar=2.0,
                in1=Fv[:, :, 0:1], op0=ALU.mult, op1=ALU.add)
            nc.vector.scalar_tensor_tensor(
                out=RWv[:, :, 127:128], in0=Fv[:, :, 126:127], scalar=2.0,
                in1=Fv[:, :, 127:128], op0=ALU.mult, op1=ALU.add)
            for c in range(NCHUNK):
                sl = slice(c * 512, (c + 1) * 512)
                t = scrapp.tile([CA, 512], BF16, tag="boxt")
                nc.vector.tensor_add(t, RW_f[:, c * 512:c * 512 + 512],
                                     RW_f[:, c * 512 + 256:c * 512 + 768])
                nc.vector.tensor_add(B_f[:, sl], t, RWc[:, sl])
            nc.vector.scalar_tensor_tensor(
                out=B_f[:, 0:128], in0=RWc[:, 128:256], scalar=2.0,
                in1=RWc[:, 0:128], op0=ALU.mult, op1=ALU.add)
            nc.vector.scalar_tensor_tensor(
                out=B_f[:, NPIX - 128:NPIX], in0=RWc[:, NPIX - 256:NPIX - 128],
                scalar=2.0, in1=RWc[:, NPIX - 128:NPIX], op0=ALU.mult, op1=ALU.add)

            # ---- phase C: channel-major MLPs
            with (
                tc.tile_pool(name="ps_y1", bufs=2, space="PSUM") as ps_y1,
                tc.tile_pool(name="ps_y2", bufs=2, space="PSUM") as ps_y2,
                tc.tile_pool(name="ps_m", bufs=2, space="PSUM") as ps_m,
                tc.tile_pool(name="ps_o", bufs=2, space="PSUM") as ps_o,
            ):
                for c in range(NCHUNK):
                    Bc = B_f[:, c * 512:(c + 1) * 512]

                    # var1' = B.T M1 B / HID  (quadratic form, [1,512] row)
                    t1 = ps_m.tile([128, 512], F32, tag="m")
                    nc.tensor.matmul(t1[0:CA, :], lhsT=m1, rhs=Bc,
                                     start=True, stop=True)
                    prod = prodp.tile([CA, 512], BF16, tag="prod")
                    nc.vector.tensor_mul(prod, t1[0:CA, :], Bc)
                    v1row = ps_m.tile([128, 512], F32, tag="m")
                    nc.tensor.matmul(v1row[0:1, :], lhsT=inv_col[0:CA, :],
                                     rhs=prod, start=True, stop=True)
                    std1row = rowsp.tile([1, 512], BF16, tag="std1")
                    nc.scalar.activation(out=std1row, in_=v1row[0:1, :],
                                         func=AF.Sqrt, bias=eps_t[0:1, :],
                                         scale=1.0)

                    # mm1 (channel-major) + x1 = relu(y1')
                    x1s = []
                    for m in range(3):
                        yps = ps_y1.tile([128, 512], F32, tag="y1")
                        nc.tensor.matmul(yps, lhsT=wc[:, m * 128:(m + 1) * 128],
                                         rhs=Bc, start=True, stop=True)
                        x1 = xsp.tile([128, 512], BF16, tag="x1")
                        if m == 0:
                            nc.vector.tensor_scalar_max(out=x1, in0=yps,
                                                        scalar1=0.0)
                        else:
                            nc.scalar.activation(out=x1, in_=yps, func=AF.Relu)
                        x1s.append(x1)

                    # mm2 + sq2 + x2 = relu(y2')
                    x2s = []
                    sqs = []
                    for m in range(3):
                        yps = ps_y2.tile([128, 512], F32, tag="y2")
                        for kb in range(3):
                            nc.tensor.matmul(
                                yps, lhsT=w2b[:, kb, m * 128:(m + 1) * 128],
                                rhs=x1s[kb], start=(kb == 0), stop=False)
                        nc.tensor.matmul(yps, lhsT=b2c[0:1, m * 128:(m + 1) * 128],
                                         rhs=std1row, start=False, stop=True)
                        sq = sqp.tile([128, 512], BF16, tag="sq")
                        nc.scalar.activation(out=sq, in_=yps, func=AF.Square,
                                             scale=SQRT_INV_HID)
                        x2 = xsp.tile([128, 512], BF16, tag="x2")
                        if m < 2:
                            nc.vector.tensor_scalar_max(out=x2, in0=yps,
                                                        scalar1=0.0)
                        else:
                            nc.scalar.activation(out=x2, in_=yps, func=AF.Relu)
                        x2s.append(x2)
                        sqs.append(sq)

                    v2row = ps_m.tile([128, 512], F32, tag="m")
                    for m in range(3):
                        nc.tensor.matmul(v2row[0:1, :], lhsT=ones_col,
                                         rhs=sqs[m], start=(m == 0),
                                         stop=(m == 2))
                    std2row = rowsp.tile([1, 512], BF16, tag="std2")
                    nc.scalar.activation(out=std2row, in_=v2row[0:1, :],
                                         func=AF.Sqrt, bias=eps_t[0:1, :],
                                         scale=1.0)

                    # rstd2 transposed to pixel-major via rank-1 matmuls
                    stdT = ps_m.tile([128, 512], F32, tag="m")
                    for i in range(4):
                        nc.tensor.matmul(
                            stdT[:, i:i + 1],
                            lhsT=std2row[0:1, i * 128:(i + 1) * 128],
                            rhs=ones_col[0:1, :], start=True, stop=True)
                    stdTc = rowsp.tile([128, 4], F32, tag="stdTc")
                    nc.vector.tensor_copy(stdTc, stdT[:, 0:4])
                    rstdT = rowsp.tile([128, 4], F32, tag="rstdT")
                    nc.vector.reciprocal(rstdT, stdTc)

                    # mm3 pixel-major + finalize (4 blocks packed in one bank)
                    o_tile = ps_o.tile([128, 512], F32, tag="o")
                    cb4 = outp.tile([128, 4, CD], F32, tag="cb")
                    nc.sync.dma_start(out=cb4,
                                      in_=dram_grp(codes, c * 512, 4, CD))
                    fin4 = outp.tile([128, 4, CD], F32, tag="fin")
                    for blk in range(4):
                        b = c * 4 + blk
                        ops = o_tile[:, blk * 128:blk * 128 + CD]
                        for kb in range(3):
                            nc.tensor.matmul(
                                ops, lhsT=x2s[kb][:, blk * 128:(blk + 1) * 128],
                                rhs=wob[:, kb, :], start=(kb == 0), stop=False)
                        nc.tensor.matmul(
                            ops, lhsT=std2row[0:1, blk * 128:(blk + 1) * 128],
                            rhs=bout_row, start=False, stop=True)
                        ot = outp.tile([128, CD], F32, tag="ot")
                        nc.scalar.activation(out=ot, in_=ops, func=AF.Copy,
                                             scale=rstdT[:, blk:blk + 1])
                        nc.vector.tensor_add(fin4[:, blk, :], ot, cb4[:, blk, :])
                    nc.sync.dma_start(out=dram_grp(out, c * 512, 4, CD),
                                      in_=fin4)

    nc.compile()
    return nc


_CACHED = {}


def kernel(**inputs) -> np.ndarray:
    codes = np.ascontiguousarray(np.asarray(inputs["codes"], dtype=np.float32))
    depth = np.ascontiguousarray(np.asarray(inputs["depth"], dtype=np.float32))
    B = codes.shape[0]
    assert codes.shape == (B, NPIX, CD) and depth.shape == (B, NPIX, DD)
    assert int(inputs["ph"]) == H and int(inputs["pw"]) == W

    ln_identity = (
        np.allclose(np.asarray(inputs["ln1_g"]), 1.0)
        and np.allclose(np.asarray(inputs["ln1_b"]), 0.0)
        and np.allclose(np.asarray(inputs["ln2_g"]), 1.0)
        and np.allclose(np.asarray(inputs["ln2_b"]), 0.0)
    )
    key = not ln_identity
    if key not in _CACHED:
        if ln_identity:
            _CACHED[key] = build_kernel_v2()
        else:
            _CACHED[key] = build_kernel(apply_ln_affine=True)
    nc = _CACHED[key]

    weights = {
        k: np.ascontiguousarray(np.asarray(inputs[k], dtype=np.float32))
        for k in ["attn_proj_w", "attn_proj_b", "attn_gate_w", "mlp_w1",
                  "mlp_b1", "ln1_g", "ln1_b", "mlp_w2", "mlp_b2", "ln2_g",
                  "ln2_b", "out_w", "out_b"]
    }
    weights["attn_gate_w"] = weights["attn_gate_w"].reshape(C, 1)

    n_cores = 8
    in_maps = []
    for core in range(n_cores):
        b = core % B
        in_maps.append({"codes": codes[b], "depth": depth[b], **weights})

    res = run_bass_kernel_spmd(nc, in_maps, core_ids=list(range(n_cores)))
    out = np.stack([res.results[core % n_cores]["out"] for core in range(B)], axis=0)
    return out.astype(np.float32)


if __name__ == "__main__":
    import reference

    inputs = reference.setup_inputs()
    expected = np.asarray(reference.reference(**inputs))
    actual = kernel(**{kk: np.asarray(v) if hasattr(v, "shape") else v
                       for kk, v in inputs.items()})
    err = np.linalg.norm(actual - expected) / np.linalg.norm(expected)
    print("Relative error:", err)



# revision 22
# speedup vs baseline: 1.8539x; 1.3957x over previous
"""Trainium2 Bass kernel for DepthAdapterWindowAttn.

Math (per batch image, H=W=128, C=106 feat channels):
  feat = concat(codes, depth)                              # (N, 106)
  s    = feat @ gate_w            (gate bias dropped: softmax-invariant)
  E    = exp(s)                   (no max-subtract needed: |s| ~ N(0,1))
  p    = feat @ Wproj + b         (proj of a shifted window == shift of proj)
  F    = [E*p ; E]                # 107 channels
  G    = box3x3_reflect(F)        # separable: W-pass then H-pass
  attended = G[0:106] / G[106]    # softmax-weighted window sum
  y1 = attended @ W1 + b1 ; x1 = relu(LN(y1))
  y2 = x1 @ W2 + b2       ; x2 = relu(LN(y2))
  out = codes + x2 @ Wout + bout

Key trick: LayerNorm is invariant to a positive per-pixel scale, so the
division by Z = G[106] cancels inside LN1:  mm1 consumes the *unnormalized*
box output G directly, with w1_aug row 106 (= b1) multiplied by the Z row,
which scales the bias by exactly the right factor.  The softmax denominator
is never divided out explicitly.

Sharding: data-parallel over batch B=8, one image per NeuronCore.
"""

import numpy as np

import concourse.bacc as bacc
import concourse.bass as bass
import concourse.mybir as mybir
import concourse.tile as tile
from concourse.bass_utils import run_bass_kernel_spmd
from concourse.masks import make_identity

F32 = mybir.dt.float32
BF16 = mybir.dt.bfloat16
AF = mybir.ActivationFunctionType
ALU = mybir.AluOpType

H = 128
W = 128
NPIX = H * W            # 16384
CD = 90                 # code dim
DD = 16                 # depth dim
C = CD + DD             # 106
CA = C + 1              # 107 (augmented with ones/E row)
HID = 384
EPS = 1e-5
NCHUNK = NPIX // 512    # 32
NBLK = NPIX // 128      # 128
GRP = 16                # LN stat batching group


def _consts(nc, tc, consts, dram, apply_ln_affine):
    """Load/cast all weights into SBUF bf16 tiles."""
    i128b = consts.tile([128, 128], BF16, tag="i128b")
    make_identity(nc, i128b)

    ones1 = consts.tile([1, 128], BF16, tag="ones1")
    nc.vector.memset(ones1, 1.0)

    eps_t = consts.tile([128, 1], F32, tag="eps_t")
    nc.vector.memset(eps_t, EPS)

    def staged(name, shape_dst, fill_zero, loads, dtype=BF16):
        stg = consts.tile(shape_dst, F32, tag=f"stg_{name}")
        if fill_zero:
            nc.vector.memset(stg, 0.0)
        for dst_sl, src_ap in loads:
            nc.sync.dma_start(out=stg[dst_sl], in_=src_ap)
        t = consts.tile(shape_dst, dtype, tag=name)
        nc.vector.tensor_copy(t, stg)
        return t

    def bcast_ap(handle, n):
        ap = handle[:]
        return bass.AP(tensor=ap.tensor, offset=ap.offset, ap=[[0, 128], [1, n]])

    k = {}
    # Wpg_aug[kin, mout]: kin 106 = ones row, mout 106 = E pass-through
    wpg = staged(
        "wpg", [CA, CA], True,
        [((slice(0, C), slice(0, C)), dram["attn_proj_w"][:, :]),
         ((slice(C, CA), slice(0, C)), dram["attn_proj_b"][None, :])])
    # E pass-through column: wpg[:, 106] = e_106, taken from the identity
    # (avoids a single-partition write at partition 106, which BIR rejects)
    nc.vector.tensor_copy(wpg[0:CA, C:C + 1], i128b[0:CA, C:C + 1])
    k["wpg"] = wpg

    k["gw_rep"] = staged(
        "gw_rep", [128, CA], True,
        [((slice(0, 128), slice(0, C)), bcast_ap(dram["attn_gate_w"], C))],
        dtype=F32)

    k["w1a"] = staged(
        "w1a", [CA, HID], False,
        [((slice(0, C), slice(0, HID)), dram["mlp_w1"][:, :]),
         ((slice(C, CA), slice(0, HID)), dram["mlp_b1"][None, :])])

    w2b_stg = consts.tile([128, 3, HID], F32, tag="w2stg")
    for kb in range(3):
        nc.sync.dma_start(out=w2b_stg[:, kb, :],
                          in_=dram["mlp_w2"][kb * 128:(kb + 1) * 128, :])
    w2b = consts.tile([128, 3, HID], BF16, tag="w2b")
    nc.vector.tensor_copy(w2b, w2b_stg)
    k["w2b"] = w2b

    wob_stg = consts.tile([128, 3, CD], F32, tag="wostg")
    for kb in range(3):
        nc.sync.dma_start(out=wob_stg[:, kb, :],
                          in_=dram["out_w"][kb * 128:(kb + 1) * 128, :])
    wob = consts.tile([128, 3, CD], BF16, tag="wob")
    nc.vector.tensor_copy(wob, wob_stg)
    k["wob"] = wob

    k["b2_row"] = staged("b2r", [1, HID], False,
                         [((slice(0, 1), slice(0, HID)), dram["mlp_b2"][None, :])])
    k["ob_row"] = staged("obr", [1, CD], False,
                         [((slice(0, 1), slice(0, CD)), dram["out_b"][None, :])])

    if apply_ln_affine:
        k["g1_rep"] = staged("g1r", [128, HID], False,
                             [((slice(0, 128), slice(0, HID)), bcast_ap(dram["ln1_g"], HID))])
        k["b1_rep"] = staged("b1r", [128, HID], False,
                             [((slice(0, 128), slice(0, HID)), bcast_ap(dram["ln1_b"], HID))])
        k["g2_rep"] = staged("g2r", [128, HID], False,
                             [((slice(0, 128), slice(0, HID)), bcast_ap(dram["ln2_g"], HID))])
        k["b2l_rep"] = staged("b2lr", [128, HID], False,
                              [((slice(0, 128), slice(0, HID)), bcast_ap(dram["ln2_b"], HID))])
    k["i128b"] = i128b
    k["ones1"] = ones1
    k["eps_t"] = eps_t
    return k


def build_kernel(apply_ln_affine: bool) -> bass.Bass:
    nc = bacc.Bacc("TRN2", target_bir_lowering=False, num_devices=8)

    dram = {}
    dram["codes"] = nc.declare_dram_parameter("codes", [NPIX, CD], F32, isOutput=False)
    dram["depth"] = nc.declare_dram_parameter("depth", [NPIX, DD], F32, isOutput=False)
    for name, shape in [
        ("attn_proj_w", [C, C]), ("attn_proj_b", [C]), ("attn_gate_w", [C, 1]),
        ("mlp_w1", [C, HID]), ("mlp_b1", [HID]), ("ln1_g", [HID]), ("ln1_b", [HID]),
        ("mlp_w2", [HID, HID]), ("mlp_b2", [HID]), ("ln2_g", [HID]), ("ln2_b", [HID]),
        ("out_w", [HID, CD]), ("out_b", [CD]),
    ]:
        dram[name] = nc.declare_dram_parameter(name, shape, F32, isOutput=False)
    out = nc.declare_dram_parameter("out", [NPIX, CD], F32, isOutput=True)
    codes = dram["codes"]
    depth = dram["depth"]

    with tile.TileContext(nc) as tc:
        with (
            tc.tile_pool(name="consts", bufs=1) as consts,
            tc.tile_pool(name="fields", bufs=1) as fields,
            tc.tile_pool(name="px", bufs=GRP + 2) as pxp,
            tc.tile_pool(name="uchunk", bufs=3) as uchp,
            tc.tile_pool(name="scrap", bufs=3) as scrapp,
            tc.tile_pool(name="ysb", bufs=GRP + 2) as ysbp,
            tc.tile_pool(name="xn", bufs=3) as xnp,
            tc.tile_pool(name="xt", bufs=3) as xtp,
            tc.tile_pool(name="stats", bufs=2) as statsp,
            tc.tile_pool(name="outp", bufs=4) as outp,
        ):
            k = _consts(nc, tc, consts, dram, apply_ln_affine)
            i128b, ones1, eps_t = k["i128b"], k["ones1"], k["eps_t"]

            # x = h*128 + w pixel flattening; free-dim pads for box shifts
            F_f = fields.tile([CA, NPIX + 2], BF16, tag="F_f")    # center off 1
            RW_f = fields.tile([CA, NPIX + 256], BF16, tag="RW_f")  # center off 128
            G_f = fields.tile([CA, NPIX], BF16, tag="G_f")
            s2dw = fields.tile([128, 128], F32, tag="s2dw")       # s[w, h]
            E2dw = fields.tile([128, 128], F32, tag="E2dw")
            nc.vector.memset(F_f[:, 0:1], 0.0)
            nc.vector.memset(F_f[:, NPIX + 1:NPIX + 2], 0.0)
            nc.vector.memset(RW_f[:, 0:128], 0.0)
            nc.vector.memset(RW_f[:, NPIX + 128:NPIX + 256], 0.0)
            Fc = F_f[:, 1:1 + NPIX]
            RWc = RW_f[:, 128:128 + NPIX]

            # ---- phase A: load, gate dot, E, u = E*feat, transpose-in, proj
            with (
                tc.tile_pool(name="ps_t", bufs=3, space="PSUM") as ps_t,
                tc.tile_pool(name="ps_f", bufs=2, space="PSUM") as ps_f,
            ):
                for g in range(NBLK // GRP):
                    pxs = []
                    for j in range(GRP):
                        b = g * GRP + j
                        px = pxp.tile([128, CA], F32, tag="px")
                        nc.sync.dma_start(out=px[:, 0:CD],
                                          in_=codes[b * 128:(b + 1) * 128, :])
                        nc.sync.dma_start(out=px[:, CD:C],
                                          in_=depth[b * 128:(b + 1) * 128, :])
                        nc.vector.memset(px[:, C:CA], 1.0)
                        scr = scrapp.tile([128, CA], F32, tag="sscr")
                        nc.vector.tensor_mul(scr, px, k["gw_rep"])
                        nc.vector.reduce_sum(s2dw[:, b:b + 1], scr,
                                             mybir.AxisListType.X)
                        pxs.append(px)
                    nc.scalar.activation(
                        out=E2dw[:, g * GRP:(g + 1) * GRP],
                        in_=s2dw[:, g * GRP:(g + 1) * GRP], func=AF.Exp)
                    for j in range(GRP):
                        b = g * GRP + j
                        upx = pxp.tile([128, CA], BF16, tag="upx")
                        nc.scalar.activation(out=upx, in_=pxs[j], func=AF.Copy,
                                             scale=E2dw[:, b:b + 1])
                        tp = ps_t.tile([CA, 128], BF16, tag="tp")
                        nc.tensor.transpose(tp, upx, i128b)
                        if b % 4 == 0:
                            uch = uchp.tile([CA, 512], BF16, tag="uc")
                        if b % 2 == 0:
                            nc.vector.tensor_copy(
                                uch[:, (b % 4) * 128:(b % 4 + 1) * 128], tp)
                        else:
                            nc.scalar.copy(
                                uch[:, (b % 4) * 128:(b % 4 + 1) * 128], tp)
                        if b % 4 == 3:
                            c = b // 4
                            fps = ps_f.tile([CA, 512], F32, tag="fps")
                            nc.tensor.matmul(fps, lhsT=k["wpg"], rhs=uch,
                                             start=True, stop=True)
                            if c % 2 == 0:
                                nc.vector.tensor_copy(Fc[:, c * 512:(c + 1) * 512], fps)
                            else:
                                nc.scalar.copy(Fc[:, c * 512:(c + 1) * 512], fps)

            # ---- phase B: separable 3x3 box with reflect boundary
            for c in range(NCHUNK):
                sl = slice(c * 512, (c + 1) * 512)
                t = scrapp.tile([CA, 512], BF16, tag="boxt")
                nc.vector.tensor_add(t, F_f[:, c * 512:c * 512 + 512],
                                     F_f[:, c * 512 + 2:c * 512 + 514])
                nc.vector.tensor_add(RWc[:, sl], t, Fc[:, sl])
            Fv = Fc.rearrange("p (h w) -> p h w", h=H)
            RWv = RWc.rearrange("p (h w) -> p h w", h=H)
            nc.vector.scalar_tensor_tensor(
                out=RWv[:, :, 0:1], in0=Fv[:, :, 1:2], scalar=2.0,
                in1=Fv[:, :, 0:1], op0=ALU.mult, op1=ALU.add)
            nc.vector.scalar_tensor_tensor(
                out=RWv[:, :, 127:128], in0=Fv[:, :, 126:127], scalar=2.0,
                in1=Fv[:, :, 127:128], op0=ALU.mult, op1=ALU.add)
            for c in range(NCHUNK):
                sl = slice(c * 512, (c + 1) * 512)
                t = scrapp.tile([CA, 512], BF16, tag="boxt")
                nc.vector.tensor_add(t, RW_f[:, c * 512:c * 512 + 512],
                                     RW_f[:, c * 512 + 256:c * 512 + 768])
                nc.vector.tensor_add(G_f[:, sl], t, RWc[:, sl])
            nc.vector.scalar_tensor_tensor(
                out=G_f[:, 0:128], in0=RWc[:, 128:256], scalar=2.0,
                in1=RWc[:, 0:128], op0=ALU.mult, op1=ALU.add)
            nc.vector.scalar_tensor_tensor(
                out=G_f[:, NPIX - 128:NPIX], in0=RWc[:, NPIX - 256:NPIX - 128],
                scalar=2.0, in1=RWc[:, NPIX - 128:NPIX], op0=ALU.mult, op1=ALU.add)

            # ---- phase C: MLPs + LNs + residual
            def ln_rstd_batched(mv):
                """mv [128, GRP, 2] (mean, var) -> rstd [128, GRP]."""
                sd = statsp.tile([128, GRP], F32, tag="sd")
                nc.scalar.activation(out=sd, in_=mv[:, :, 1], func=AF.Sqrt,
                                     bias=eps_t, scale=1.0)
                rstd = statsp.tile([128, GRP], F32, tag="rstd")
                nc.vector.reciprocal(rstd, sd)
                return rstd

            def neg_mu_rstd(mv, rstd):
                """[128, GRP] tile of -mean * rstd (fused LN bias)."""
                nm = statsp.tile([128, GRP], F32, tag="nm")
                nc.vector.scalar_tensor_tensor(
                    out=nm, in0=mv[:, :, 0], scalar=-1.0, in1=rstd,
                    op0=ALU.mult, op1=ALU.mult)
                return nm

            def apply_ln_relu(xn, y_sb, mv, nm, rstd, j, gamma_rep, beta_rep):
                """x = relu(LN(y)): one fused scalar-engine op when the LN
                affine is identity (relu commutes with the positive rstd
                scale: Relu(y*rstd - mu*rstd) == relu(LN(y)))."""
                if gamma_rep is None:
                    nc.scalar.activation(
                        out=xn, in_=y_sb, func=AF.Relu,
                        bias=nm[:, j:j + 1], scale=rstd[:, j:j + 1])
                else:
                    nc.vector.tensor_scalar(
                        out=xn, in0=y_sb, scalar1=mv[:, j, 0:1],
                        scalar2=rstd[:, j:j + 1],
                        op0=ALU.subtract, op1=ALU.mult)
                    nc.vector.tensor_mul(xn, xn, gamma_rep)
                    nc.vector.tensor_add(xn, xn, beta_rep)
                    nc.scalar.activation(out=xn, in_=xn, func=AF.Relu)

            with (
                tc.tile_pool(name="ps_y", bufs=2, space="PSUM") as ps_y,
                tc.tile_pool(name="ps_xt", bufs=2, space="PSUM") as ps_xt,
                tc.tile_pool(name="ps_o", bufs=2, space="PSUM") as ps_o,
            ):
                def transpose_copy(xn):
                    """xn (already relu'd) -> 3 transposed [128,128] tiles."""
                    xt = xtp.tile([128, 3, 128], BF16, tag="xt")
                    tps = ps_xt.tile([128, 3, 128], BF16, tag="tps")
                    for kb in range(3):
                        nc.tensor.transpose(tps[:, kb, :],
                                            xn[:, kb * 128:(kb + 1) * 128], i128b)
                        if kb == 1:
                            nc.scalar.copy(xt[:, kb, :], tps[:, kb, :])
                        else:
                            nc.vector.tensor_copy(xt[:, kb, :], tps[:, kb, :])
                    return xt

                for g in range(NBLK // GRP):
                    mv1 = statsp.tile([128, GRP, 2], F32, tag="mv1")
                    y1s = []
                    for j in range(GRP):
                        b = g * GRP + j
                        yps = ps_y.tile([128, HID], F32, tag="yps")
                        nc.tensor.matmul(yps, lhsT=G_f[:, b * 128:(b + 1) * 128],
                                         rhs=k["w1a"], start=True, stop=True)
                        y_sb = ysbp.tile([128, HID], BF16, tag="y1sb")
                        nc.scalar.copy(y_sb, yps)
                        st = scrapp.tile([128, 6], F32, tag="st")
                        nc.vector.bn_stats(out=st, in_=yps)
                        nc.vector.bn_aggr(out=mv1[:, j, :], in_=st)
                        y1s.append(y_sb)
                    rstd1 = ln_rstd_batched(mv1)
                    nm1 = neg_mu_rstd(mv1, rstd1)

                    mv2 = statsp.tile([128, GRP, 2], F32, tag="mv2")
                    y2s = []
                    for j in range(GRP):
                        b = g * GRP + j
                        xn = xnp.tile([128, HID], BF16, tag="x1n")
                        apply_ln_relu(xn, y1s[j], mv1, nm1, rstd1, j,
                                      k.get("g1_rep"), k.get("b1_rep"))
                        xt = transpose_copy(xn)
                        yps = ps_y.tile([128, HID], F32, tag="yps")
                        for kb in range(3):
                            nc.tensor.matmul(yps, lhsT=xt[:, kb, :],
                                             rhs=k["w2b"][:, kb, :],
                                             start=(kb == 0), stop=False)
                        nc.tensor.matmul(yps, lhsT=ones1, rhs=k["b2_row"],
                                         start=False, stop=True)
                        y_sb = ysbp.tile([128, HID], BF16, tag="y2sb")
                        nc.scalar.copy(y_sb, yps)
                        st = scrapp.tile([128, 6], F32, tag="st")
                        nc.vector.bn_stats(out=st, in_=yps)
                        nc.vector.bn_aggr(out=mv2[:, j, :], in_=st)
                        y2s.append(y_sb)
                    rstd2 = ln_rstd_batched(mv2)
                    nm2 = neg_mu_rstd(mv2, rstd2)

                    for j in range(GRP):
                        b = g * GRP + j
                        xn = xnp.tile([128, HID], BF16, tag="x2n")
                        apply_ln_relu(xn, y2s[j], mv2, nm2, rstd2, j,
                                      k.get("g2_rep"), k.get("b2l_rep"))
                        xt = transpose_copy(xn)
                        ops = ps_o.tile([128, CD], F32, tag="ops")
                        for kb in range(3):
                            nc.tensor.matmul(ops, lhsT=xt[:, kb, :],
                                             rhs=k["wob"][:, kb, :],
                                             start=(kb == 0), stop=False)
                        nc.tensor.matmul(ops, lhsT=ones1, rhs=k["ob_row"],
                                         start=False, stop=True)
                        cb = outp.tile([128, CD], F32, tag="cb")
                        nc.sync.dma_start(out=cb,
                                          in_=codes[b * 128:(b + 1) * 128, :])
                        ot = outp.tile([128, CD], F32, tag="ot")
                        nc.vector.tensor_add(ot, ops, cb)
                        nc.sync.dma_start(out=out[b * 128:(b + 1) * 128, :], in_=ot)

    nc.compile()
    return nc


def build_kernel_v2() -> bass.Bass:
    """Channel-major phase C: no per-block transposes, matmul-based LN stats.

    Tricks on top of v1:
      - Wpg folds into W1:  y1 = (Wpg @ W1aug).T @ box(u), so the proj matmul
        and the F field disappear; the box filter runs directly on
        u = [E*feat ; E].
      - Column-centered weights: subtracting each row's mean over the output
        dim from Wc / W2 / b2 makes mean(y) == 0 exactly, killing the LN mean
        computation (LN is then y * rstd).
      - Channel-major mm1/mm2: LN variance via sum of squares over the
        partition dim = ones-vector matmuls; var1 via the quadratic form
        B.T (Wc' Wc'.T) B (one [107x107] matmul + one dot pass).
      - Deferred normalization: x1 = relu(y1') unscaled (LN2 is invariant to
        positive per-pixel scale); b2 rides a rank-1 matmul scaled by std1;
        the exact rstd2 scale is applied pixel-major on the 90-dim output.
    """
    nc = bacc.Bacc("TRN2", target_bir_lowering=False, num_devices=8)

    dram = {}
    dram["codes"] = nc.declare_dram_parameter("codes", [NPIX, CD], F32, isOutput=False)
    dram["depth"] = nc.declare_dram_parameter("depth", [NPIX, DD], F32, isOutput=False)
    for name, shape in [
        ("attn_proj_w", [C, C]), ("attn_proj_b", [C]), ("attn_gate_w", [C, 1]),
        ("mlp_w1", [C, HID]), ("mlp_b1", [HID]), ("ln1_g", [HID]), ("ln1_b", [HID]),
        ("mlp_w2", [HID, HID]), ("mlp_b2", [HID]), ("ln2_g", [HID]), ("ln2_b", [HID]),
        ("out_w", [HID, CD]), ("out_b", [CD]),
    ]:
        dram[name] = nc.declare_dram_parameter(name, shape, F32, isOutput=False)
    out = nc.declare_dram_parameter("out", [NPIX, CD], F32, isOutput=True)
    codes = dram["codes"]
    depth = dram["depth"]
    INV_HID = 1.0 / HID
    SQRT_INV_HID = float(np.sqrt(1.0 / HID))

    with tile.TileContext(nc) as tc:
        with (
            tc.tile_pool(name="consts", bufs=1) as consts,
            tc.tile_pool(name="fields", bufs=1) as fields,
            tc.tile_pool(name="px", bufs=2) as pxp,
            tc.tile_pool(name="upx", bufs=6) as upxp,
            tc.tile_pool(name="scrap", bufs=4) as scrapp,
            tc.tile_pool(name="xs", bufs=9) as xsp,
            tc.tile_pool(name="sq", bufs=6) as sqp,
            tc.tile_pool(name="prod", bufs=3) as prodp,
            tc.tile_pool(name="rows", bufs=4) as rowsp,
            tc.tile_pool(name="outp", bufs=8) as outp,
        ):
            # ---- constants / weight prep
            i128b = consts.tile([128, 128], BF16, tag="i128b")
            make_identity(nc, i128b)
            ones_col = consts.tile([128, 1], BF16, tag="ones_col")
            nc.vector.memset(ones_col, 1.0)
            inv_col = consts.tile([128, 1], BF16, tag="inv_col")
            nc.vector.memset(inv_col, INV_HID)
            eps_t = consts.tile([128, 1], F32, tag="eps_t")
            nc.vector.memset(eps_t, EPS)

            def staged(name, shape_dst, fill_zero, loads, dtype=BF16):
                stg = consts.tile(shape_dst, F32, tag=f"stg_{name}")
                if fill_zero:
                    nc.vector.memset(stg, 0.0)
                for dst_sl, src_ap in loads:
                    nc.sync.dma_start(out=stg[dst_sl], in_=src_ap)
                t = consts.tile(shape_dst, dtype, tag=name)
                nc.vector.tensor_copy(t, stg)
                return t

            def bcast_ap(handle, n):
                ap = handle[:]
                return bass.AP(tensor=ap.tensor, offset=ap.offset,
                               ap=[[0, 128], [1, n]])

            wpg = staged(
                "wpg", [CA, CA], True,
                [((slice(0, C), slice(0, C)), dram["attn_proj_w"][:, :]),
                 ((slice(C, CA), slice(0, C)), dram["attn_proj_b"][None, :])])
            nc.vector.tensor_copy(wpg[0:CA, C:C + 1], i128b[0:CA, C:C + 1])

            gw_rep = staged(
                "gw_rep", [128, CA], True,
                [((slice(0, 128), slice(0, C)), bcast_ap(dram["attn_gate_w"], C))],
                dtype=F32)

            w1a = staged(
                "w1a", [CA, HID], False,
                [((slice(0, C), slice(0, HID)), dram["mlp_w1"][:, :]),
                 ((slice(C, CA), slice(0, HID)), dram["mlp_b1"][None, :])])

            with tc.tile_pool(name="ps_prep", bufs=2, space="PSUM") as ps_prep:
                # Wc = rowcenter(Wpg @ W1aug)  [CA, HID]
                tp_ps = ps_prep.tile([128, 128], BF16, tag="prepb")
                nc.tensor.transpose(tp_ps[0:CA, 0:CA], wpg, i128b[0:CA, 0:CA])
                wpgT = consts.tile([CA, CA], BF16, tag="wpgT")
                nc.vector.tensor_copy(wpgT, tp_ps[0:CA, 0:CA])
                cb_ps = ps_prep.tile([128, 512], F32, tag="prepf")
                nc.tensor.matmul(cb_ps[0:CA, 0:HID], lhsT=wpgT, rhs=w1a,
                                 start=True, stop=True)
                rs = consts.tile([CA, 1], F32, tag="rs")
                nc.vector.reduce_sum(rs, cb_ps[0:CA, 0:HID], mybir.AxisListType.X)
                rm = consts.tile([CA, 1], F32, tag="rm")
                nc.vector.tensor_scalar_mul(rm, rs, INV_HID)
                wc = consts.tile([CA, HID], BF16, tag="wc")
                nc.vector.tensor_scalar_sub(wc, cb_ps[0:CA, 0:HID], rm)

                # M1 = Wc @ Wc.T  [CA, CA]
                wcT = consts.tile([128, 3, CA], BF16, tag="wcT")
                for kb in range(3):
                    t_ps = ps_prep.tile([128, 128], BF16, tag="prepb")
                    nc.tensor.transpose(t_ps[0:128, 0:CA],
                                        wc[:, kb * 128:(kb + 1) * 128],
                                        i128b[0:CA, 0:CA])
                    nc.vector.tensor_copy(wcT[:, kb, :], t_ps[0:128, 0:CA])
                m1_ps = ps_prep.tile([128, 512], F32, tag="prepf")
                for kb in range(3):
                    nc.tensor.matmul(m1_ps[0:CA, 0:CA], lhsT=wcT[:, kb, :],
                                     rhs=wcT[:, kb, :], start=(kb == 0),
                                     stop=(kb == 2))
                m1 = consts.tile([CA, CA], BF16, tag="m1")
                nc.vector.tensor_copy(m1, m1_ps[0:CA, 0:CA])

            # W2 row-centered over output dim, [128, 3, HID]
            w2stg = consts.tile([128, 3, HID], F32, tag="w2stg")
            for kb in range(3):
                nc.sync.dma_start(out=w2stg[:, kb, :],
                                  in_=dram["mlp_w2"][kb * 128:(kb + 1) * 128, :])
            w2b = consts.tile([128, 3, HID], BF16, tag="w2b")
            for kb in range(3):
                rs2 = consts.tile([128, 1], F32, tag="rs2")
                nc.vector.reduce_sum(rs2, w2stg[:, kb, :], mybir.AxisListType.X)
                rm2 = consts.tile([128, 1], F32, tag="rm2")
                nc.vector.tensor_scalar_mul(rm2, rs2, INV_HID)
                nc.vector.tensor_scalar_sub(w2b[:, kb, :], w2stg[:, kb, :], rm2)

            # b2 centered, [1, HID]
            b2stg = consts.tile([1, HID], F32, tag="b2stg")
            nc.sync.dma_start(out=b2stg, in_=dram["mlp_b2"][None, :])
            b2s = consts.tile([1, 1], F32, tag="b2s")
            nc.vector.reduce_sum(b2s, b2stg, mybir.AxisListType.X)
            b2m = consts.tile([1, 1], F32, tag="b2m")
            nc.vector.tensor_scalar_mul(b2m, b2s, INV_HID)
            b2c = consts.tile([1, HID], BF16, tag="b2c")
            nc.vector.tensor_scalar_sub(b2c, b2stg, b2m)

            wob_stg = consts.tile([128, 3, CD], F32, tag="wostg")
            for kb in range(3):
                nc.sync.dma_start(out=wob_stg[:, kb, :],
                                  in_=dram["out_w"][kb * 128:(kb + 1) * 128, :])
            wob = consts.tile([128, 3, CD], BF16, tag="wob")
            nc.vector.tensor_copy(wob, wob_stg)
            boutrep4 = consts.tile([128, 4, CD], F32, tag="boutrep4")
            for i in range(4):
                nc.sync.dma_start(out=boutrep4[:, i, :],
                                  in_=bcast_ap(dram["out_b"], CD))

            # ---- fields
            F_f = fields.tile([CA, NPIX + 2], BF16, tag="F_f")
            RW_f = fields.tile([CA, NPIX + 256], BF16, tag="RW_f")
            B_f = fields.tile([CA, NPIX], BF16, tag="B_f")
            s2dw = fields.tile([128, 128], F32, tag="s2dw")
            E2dw = fields.tile([128, 128], F32, tag="E2dw")
            nc.vector.memset(F_f[:, 0:1], 0.0)
            nc.vector.memset(F_f[:, NPIX + 1:NPIX + 2], 0.0)
            nc.vector.memset(RW_f[:, 0:128], 0.0)
            nc.vector.memset(RW_f[:, NPIX + 128:NPIX + 256], 0.0)
            Fc = F_f[:, 1:1 + NPIX]
            RWc = RW_f[:, 128:128 + NPIX]

            # ---- phase A: load, gate dot, E, u = E*feat_aug, transpose into Fc
            def dram_grp(handle, row0, nblk, ncol):
                """3D AP over DRAM rows [row0, row0+nblk*128) as [128, nblk, ncol]."""
                ap = handle[:]
                return bass.AP(tensor=ap.tensor, offset=ap.offset + row0 * ncol,
                               ap=[[ncol, 128], [128 * ncol, nblk], [1, ncol]])

            with tc.tile_pool(name="ps_t", bufs=3, space="PSUM") as ps_t:
                for g in range(NBLK // GRP):
                    pxg = pxp.tile([128, GRP, CA], F32, tag="px")
                    nc.sync.dma_start(out=pxg[:, :, 0:CD],
                                      in_=dram_grp(codes, g * GRP * 128, GRP, CD))
                    nc.sync.dma_start(out=pxg[:, :, CD:C],
                                      in_=dram_grp(depth, g * GRP * 128, GRP, DD))
                    nc.vector.memset(pxg[:, :, C:CA], 1.0)
                    for j in range(GRP):
                        b = g * GRP + j
                        scr = scrapp.tile([128, CA], F32, tag="gscr")
                        nc.vector.tensor_mul(scr, pxg[:, j, :], gw_rep)
                        nc.vector.reduce_sum(s2dw[:, b:b + 1], scr,
                                             mybir.AxisListType.X)
                    nc.scalar.activation(
                        out=E2dw[:, g * GRP:(g + 1) * GRP],
                        in_=s2dw[:, g * GRP:(g + 1) * GRP], func=AF.Exp)
                    for j in range(GRP):
                        b = g * GRP + j
                        upx = upxp.tile([128, CA], BF16, tag="upx")
                        nc.scalar.activation(out=upx, in_=pxg[:, j, :],
                                             func=AF.Copy,
                                             scale=E2dw[:, b:b + 1])
                        tp = ps_t.tile([CA, 128], BF16, tag="tp")
                        nc.tensor.transpose(tp, upx, i128b)
                        nc.vector.tensor_copy(Fc[:, b * 128:(b + 1) * 128], tp)

            # ---- phase B: separable 3x3 box with reflect boundary -> B_f
            for c in range(NCHUNK):
                sl = slice(c * 512, (c + 1) * 512)
                t = scrapp.tile([CA, 512], BF16, tag="boxt")
                nc.vector.tensor_add(t, F_f[:, c * 512:c * 512 + 512],
                                     F_f[:, c * 512 + 2:c * 512 + 514])
                nc.vector.tensor_add(RWc[:, sl], t, Fc[:, sl])
            Fv = Fc.rearrange("p (h w) -> p h w", h=H)
            RWv = RWc.rearrange("p (h w) -> p h w", h=H)
            nc.vector.scalar_tensor_tensor(
                out=RWv[:, :, 0:1], in0=Fv[:, :, 1:2], scalar=2.0,
                in1=Fv[:, :, 0:1], op0=ALU.mult, op1=ALU.add)
            nc.vector.scalar_tensor_tensor(
                out=RWv[:, :, 127:128], in0=Fv[:, :, 126:127], scalar=2.0,
                in1=Fv[:, :, 127:128], op0=ALU.mult, op1=ALU.add)
            for c in range(NCHUNK):
                sl = slice(c * 512, (c + 1) * 512)
                t = scrapp.tile([CA, 512], BF16, tag="boxt")
                nc.vector.tensor_add(t, RW_f[:, c * 512:c * 512 + 512],
                                     RW_f[:, c * 512 + 256:c * 512 + 768])
                nc.vector.tensor_add(B_f[:, sl], t, RWc[:, sl])
            nc.vector.scalar_tensor_tensor(
                out=B_f[:, 0:128], in0=RWc[:, 128:256], scalar=2.0,
                in1=RWc[:, 0:128], op0=ALU.mult, op1=ALU.add)
            nc.vector.scalar_tensor_tensor(
                out=B_f[:, NPIX - 128:NPIX], in0=RWc[:, NPIX - 256:NPIX - 128],
                scalar=2.0, in1=RWc[:, NPIX - 128:NPIX], op0=ALU.mult, op1=ALU.add)

            # ---- phase C: channel-major MLPs
            with (
                tc.tile_pool(name="ps_y1", bufs=2, space="PSUM") as ps_y1,
                tc.tile_pool(name="ps_y2", bufs=2, space="PSUM") as ps_y2,
                tc.tile_pool(name="ps_m", bufs=2, space="PSUM") as ps_m,
                tc.tile_pool(name="ps_o", bufs=2, space="PSUM") as ps_o,
            ):
                for c in range(NCHUNK):
                    Bc = B_f[:, c * 512:(c + 1) * 512]

                    # var1' = B.T M1 B / HID  (quadratic form, [1,512] row)
                    t1 = ps_m.tile([128, 512], F32, tag="m")
                    nc.tensor.matmul(t1[0:CA, :], lhsT=m1, rhs=Bc,
                                     start=True, stop=True)
                    prod = prodp.tile([CA, 512], BF16, tag="prod")
                    nc.vector.tensor_mul(prod, t1[0:CA, :], Bc)
                    v1row = ps_m.tile([128, 512], F32, tag="m")
                    nc.tensor.matmul(v1row[0:1, :], lhsT=inv_col[0:CA, :],
                                     rhs=prod, start=True, stop=True)
                    std1row = rowsp.tile([1, 512], BF16, tag="std1")
                    nc.scalar.activation(out=std1row, in_=v1row[0:1, :],
                                         func=AF.Sqrt, bias=eps_t[0:1, :],
                                         scale=1.0)

                    # mm1 (channel-major) + x1 = relu(y1')
                    x1s = []
                    for m in range(3):
                        yps = ps_y1.tile([128, 512], F32, tag="y1")
                        nc.tensor.matmul(yps, lhsT=wc[:, m * 128:(m + 1) * 128],
                                         rhs=Bc, start=True, stop=True)
                        x1 = xsp.tile([128, 512], BF16, tag="x1")
                        if m == 0:
                            nc.vector.tensor_scalar_max(out=x1, in0=yps,
                                                        scalar1=0.0)
                        else:
                            nc.scalar.activation(out=x1, in_=yps, func=AF.Relu)
                        x1s.append(x1)

                    # mm2 + sq2 + x2 = relu(y2')
                    x2s = []
                    sqs = []
                    for m in range(3):
                        yps = ps_y2.tile([128, 512], F32, tag="y2")
                        for kb in range(3):
                            nc.tensor.matmul(
                                yps, lhsT=w2b[:, kb, m * 128:(m + 1) * 128],
                                rhs=x1s[kb], start=(kb == 0), stop=False)
                        nc.tensor.matmul(yps, lhsT=b2c[0:1, m * 128:(m + 1) * 128],
                                         rhs=std1row, start=False, stop=True)
                        sq = sqp.tile([128, 512], BF16, tag="sq")
                        nc.scalar.activation(out=sq, in_=yps, func=AF.Square,
                                             scale=SQRT_INV_HID)
                        x2 = xsp.tile([128, 512], BF16, tag="x2")
                        if m < 2:
                            nc.vector.tensor_scalar_max(out=x2, in0=yps,
                                                        scalar1=0.0)
                        else:
                            nc.scalar.activation(out=x2, in_=yps, func=AF.Relu)
                        x2s.append(x2)
                        sqs.append(sq)

                    # var2 pixel-major: varT[px, blk] = sum_ch sq[ch, px]
                    o_tile = ps_o.tile([128, 512], F32, tag="o")
                    for blk in range(4):
                        for kb in range(3):
                            nc.tensor.matmul(
                                o_tile[:, 508 + blk:509 + blk],
                                lhsT=sqs[kb][:, blk * 128:(blk + 1) * 128],
                                rhs=ones_col, start=(kb == 0), stop=(kb == 2))
                    vc = rowsp.tile([128, 4], F32, tag="vc")
                    nc.vector.tensor_copy(vc, o_tile[:, 508:512])
                    stds = rowsp.tile([128, 4], F32, tag="stds")
                    nc.scalar.activation(out=stds, in_=vc, func=AF.Sqrt,
                                         bias=eps_t, scale=1.0)
                    rstdT = rowsp.tile([128, 4], F32, tag="rstdT")
                    nc.vector.reciprocal(rstdT, stds)

                    # mm3 pixel-major + finalize (4 blocks packed in one bank)
                    cb4 = outp.tile([128, 4, CD], F32, tag="cb")
                    nc.sync.dma_start(out=cb4,
                                      in_=dram_grp(codes, c * 512, 4, CD))
                    cbb4 = outp.tile([128, 4, CD], F32, tag="cbb")
                    nc.vector.tensor_add(cbb4, cb4, boutrep4)
                    fin4 = outp.tile([128, 4, CD], F32, tag="fin")
                    for blk in range(4):
                        b = c * 4 + blk
                        ops = o_tile[:, blk * 128:blk * 128 + CD]
                        for kb in range(3):
                            nc.tensor.matmul(
                                ops, lhsT=x2s[kb][:, blk * 128:(blk + 1) * 128],
                                rhs=wob[:, kb, :], start=(kb == 0),
                                stop=(kb == 2))
                        ot = outp.tile([128, CD], F32, tag="ot")
                        nc.scalar.activation(out=ot, in_=ops, func=AF.Copy,
                                             scale=rstdT[:, blk:blk + 1])
                        nc.vector.tensor_add(fin4[:, blk, :], ot,
                                             cbb4[:, blk, :])
                    nc.sync.dma_start(out=dram_grp(out, c * 512, 4, CD),
                                      in_=fin4)

    nc.compile()
    return nc


_CACHED = {}


def kernel(**inputs) -> np.ndarray:
    codes = np.ascontiguousarray(np.asarray(inputs["codes"], dtype=np.float32))
    depth = np.ascontiguousarray(np.asarray(inputs["depth"], dtype=np.float32))
    B = codes.shape[0]
    assert codes.shape == (B, NPIX, CD) and depth.shape == (B, NPIX, DD)
    assert int(inputs["ph"]) == H and int(inputs["pw"]) == W

    ln_identity = (
        np.allclose(np.asarray(inputs["ln1_g"]), 1.0)
        and np.allclose(np.asarray(inputs["ln1_b"]), 0.0)
        and np.allclose(np.asarray(inputs["ln2_g"]), 1.0)
        and np.allclose(np.asarray(inputs["ln2_b"]), 0.0)
    )
    key = not ln_identity
    if key not in _CACHED:
        if ln_identity:
            _CACHED[key] = build_kernel_v2()
        else:
            _CACHED[key] = build_kernel(apply_ln_affine=True)
    nc = _CACHED[key]

    weights = {
        k: np.ascontiguousarray(np.asarray(inputs[k], dtype=np.float32))
        for k in ["attn_proj_w", "attn_proj_b", "attn_gate_w", "mlp_w1",
                  "mlp_b1", "ln1_g", "ln1_b", "mlp_w2", "mlp_b2", "ln2_g",
                  "ln2_b", "out_w", "out_b"]
    }
    weights["attn_gate_w"] = weights["attn_gate_w"].reshape(C, 1)

    n_cores = 8
    in_maps = []
    for core in range(n_cores):
        b = core % B
        in_maps.append({"codes": codes[b], "depth": depth[b], **weights})

    res = run_bass_kernel_spmd(nc, in_maps, core_ids=list(range(n_cores)))
    out = np.stack([res.results[core % n_cores]["out"] for core in range(B)], axis=0)
    return out.astype(np.float32)


if __name__ == "__main__":
    import reference

    inputs = reference.setup_inputs()
    expected = np.asarray(reference.reference(**inputs))
    actual = kernel(**{kk: np.asarray(v) if hasattr(v, "shape") else v
                       for kk, v in inputs.items()})
    err = np.linalg.norm(actual - expected) / np.linalg.norm(expected)
    print("Relative error:", err)

